# revision 1
# baseline (speedup 1.0000x reference)
# Trainium2 Bass kernel for nn_LocalAggregator (Gaussian -> voxel-grid semantic
# compositing).  Strategy: data-parallel over the N=129600 query points (8 cores
# x 16200 points).  Each core's slab is split into 45 groups of 360 points (10
# z-columns of 36).  For every group the host builds the exact list of Gaussians
# whose integer box overlaps the group's (x, y) column rectangle; the quadratic
# form, log-opacity, and the x/y/z integer box tests (as one-hot penalty rows)
# are all folded into a single fp32 matmul
#     E[g, n] = coef[56, G].T @ feat[56, 360]
# followed by Exp on the scalar engine and an fp16 matmul with the semantics to
# produce the [13, 360] output block.  No per-pair vector ops at all.
import numpy as np
import ml_dtypes

H, W, D = 60, 60, 36
GRID = 0.08
SCALE_MULT = 3.0
P = 2048
C = 13
N = H * W * D                  # 129600
NCORES = 8
NPC = N // NCORES              # 16200 points per core
GCOLS = 10                     # columns per group
GPTS = GCOLS * D               # 360 points per group
NG = NPC // GPTS               # 45 groups per core
KF = 10 + GCOLS + D            # 56 feature rows
PEN = -2000.0                  # box-miss penalty (exp(PEN) == 0 in fp32)

_NC_CACHE: dict = {}
_JIT_CACHE: dict = {}


def _build_nc(L_slots, use_f32r=False):
    import concourse.bacc as bacc
    import concourse.tile as tile
    from concourse import mybir

    Ltot = int(np.sum(L_slots))
    offs = np.concatenate([[0], np.cumsum(L_slots)]).astype(int)
    units = []
    for g in range(NG):
        L = int(L_slots[g])
        for s in range(0, L, 128):
            units.append((g, int(offs[g]) + s, min(128, L - s)))
    NU = len(units)

    nc = bacc.Bacc("TRN2", target_bir_lowering=False, debug=False,
                   num_devices=NCORES)
    f32 = mybir.dt.float32
    fmm = mybir.dt.float32r if use_f32r else mybir.dt.float32
    f16 = mybir.dt.float16
    RHS = nc.dram_tensor("RHS", [KF, NPC], fmm, kind="ExternalInput")
    COEF = nc.dram_tensor("COEF", [KF, Ltot], fmm, kind="ExternalInput")
    SEMP = nc.dram_tensor("SEMP", [128, NU * C], f16, kind="ExternalInput")
    OUT = nc.dram_tensor("OUT", [C, NPC], f32, kind="ExternalOutput")

    with tile.TileContext(nc) as tc:
        with (
            tc.tile_pool(name="big", bufs=1) as big_pool,
            tc.tile_pool(name="w", bufs=4) as w_pool,
            tc.tile_pool(name="psE", bufs=4, space="PSUM") as pse_pool,
            tc.tile_pool(name="psO", bufs=2, space="PSUM") as pso_pool,
        ):
            rhs_b = big_pool.tile([KF, NPC], fmm)
            coef_b = big_pool.tile([KF, Ltot], fmm)
            semp_b = big_pool.tile([128, NU * C], f16)
            out_b = big_pool.tile([C, NPC], f32)
            # chunked loads so compute can start after the first slice lands
            NCH = 5
            for ch in range(NCH):
                g0, g1 = ch * NG // NCH, (ch + 1) * NG // NCH
                a, b = g0 * GPTS, g1 * GPTS
                nc.sync.dma_start(rhs_b[:, a:b], RHS[:, a:b])
                a, b = int(offs[g0]), int(offs[g1])
                nc.sync.dma_start(coef_b[:, a:b], COEF[:, a:b])
                u0 = sum(1 for (g, _, _) in units if g < g0)
                u1 = sum(1 for (g, _, _) in units if g < g1)
                nc.sync.dma_start(semp_b[:, u0 * C:u1 * C],
                                  SEMP[:, u0 * C:u1 * C])
            ti_prev = -1
            for u, (g, off, Mt) in enumerate(units):
                first = (u == 0) or (units[u - 1][0] != g)
                last = (u == NU - 1) or (units[u + 1][0] != g)
                if first:
                    psO = pso_pool.tile([C, GPTS], f32)
                psE = pse_pool.tile([Mt, GPTS], f32)
                nc.tensor.matmul(psE[:], coef_b[:, off:off + Mt],
                                 rhs_b[:, g * GPTS:(g + 1) * GPTS],
                                 start=True, stop=True)
                w_t = w_pool.tile([Mt, GPTS], f16)
                nc.scalar.activation(w_t[:], psE[:],
                                     mybir.ActivationFunctionType.Exp)
                nc.tensor.matmul(psO[:], semp_b[0:Mt, u * C:(u + 1) * C],
                                 w_t[:], start=first, stop=last)
                if last:
                    nc.vector.tensor_copy(out_b[:, g * GPTS:(g + 1) * GPTS],
                                          psO[:])
            nc.sync.dma_start(OUT[:], out_b[:])
    nc.compile()
    return nc


def _get_nc(L_slots, use_f32r=False):
    key = (tuple(int(x) for x in L_slots), use_f32r)
    if key not in _NC_CACHE:
        _NC_CACHE[key] = _build_nc(L_slots, use_f32r=use_f32r)
    return _NC_CACHE[key]


def _get_runner(nc):
    """Cached shard_map-jitted executor for one Bass program (axon/PJRT path).

    Mirrors concourse.bass2jax.run_bass_via_pjrt but keeps the jitted callable
    so repeated runs don't rebuild/recompile."""
    if id(nc) in _JIT_CACHE:
        return _JIT_CACHE[id(nc)]
    import jax
    from concourse import bass2jax, mybir
    from jax.experimental.shard_map import shard_map
    from jax.sharding import Mesh, PartitionSpec

    bass2jax.install_neuronx_cc_hook()
    partition_name = (nc.partition_id_tensor.name
                      if nc.partition_id_tensor else None)
    in_names, out_names, out_avals, zero_outs = [], [], [], []
    for alloc in nc.m.functions[0].allocations:
        if not isinstance(alloc, mybir.MemoryLocationSet):
            continue
        name = alloc.memorylocations[0].name
        if alloc.kind == "ExternalInput":
            if name == partition_name:
                continue
            in_names.append(name)
        elif alloc.kind == "ExternalOutput":
            shape = tuple(alloc.tensor_shape)
            dtype = mybir.dt.np(alloc.dtype)
            out_names.append(name)
            out_avals.append(jax.core.ShapedArray(shape, dtype))
            zero_outs.append(np.zeros(shape, dtype))
    n_params = len(in_names)
    all_in_names = in_names + out_names
    if partition_name is not None:
        all_in_names = all_in_names + [partition_name]

    def _body(*args):
        operands = list(args)
        if partition_name is not None:
            operands.append(bass2jax.partition_id_tensor())
        outs = bass2jax._bass_exec_p.bind(
            *operands,
            out_avals=tuple(out_avals),
            in_names=tuple(all_in_names),
            out_names=tuple(out_names),
            lowering_input_output_aliases=(),
            sim_require_finite=True,
            sim_require_nnan=True,
            nc=nc,
        )
        return tuple(outs)

    devices = jax.devices()[:NCORES]
    mesh = Mesh(np.asarray(devices), ("core",))
    donate = tuple(range(n_params, n_params + len(out_names)))
    sharded = jax.jit(
        shard_map(_body, mesh=mesh,
                  in_specs=(PartitionSpec("core"),) * (n_params + len(out_names)),
                  out_specs=(PartitionSpec("core"),) * len(out_names),
                  check_rep=False),
        donate_argnums=donate, keep_unused=True)

    def run(in_maps, rounds=1):
        concat_in = [np.concatenate([np.asarray(m[nm]) for m in in_maps], axis=0)
                     for nm in in_names]
        outs = None
        for _ in range(rounds):
            zo = [np.concatenate([z] * NCORES, axis=0) for z in zero_outs]
            outs = sharded(*concat_in, *zo)
        outs = [np.asarray(o) for o in outs]
        results = []
        for ci in range(NCORES):
            d = {}
            for oi, nm in enumerate(out_names):
                per = outs[oi].shape[0] // NCORES
                d[nm] = outs[oi][ci * per:(ci + 1) * per]
            results.append(d)
        return results, sharded, (concat_in, zero_outs, in_names, out_names)

    sharded_nd = jax.jit(
        shard_map(_body, mesh=mesh,
                  in_specs=(PartitionSpec("core"),) * (n_params + len(out_names)),
                  out_specs=(PartitionSpec("core"),) * len(out_names),
                  check_rep=False),
        keep_unused=True)

    def timeit(in_maps, iters=30):
        import time as _time
        from jax.sharding import NamedSharding
        sh = NamedSharding(mesh, PartitionSpec("core"))
        concat_in = [np.concatenate([np.asarray(m[nm]) for m in in_maps], axis=0)
                     for nm in in_names]
        zo = [np.concatenate([z] * NCORES, axis=0) for z in zero_outs]
        args = [jax.device_put(a, sh) for a in concat_in + zo]
        outs = sharded_nd(*args)
        jax.block_until_ready(outs)
        t0 = _time.time()
        for _ in range(iters):
            outs = sharded_nd(*args)
        jax.block_until_ready(outs)
        return (_time.time() - t0) / iters

    run.timeit = timeit
    _JIT_CACHE[id(nc)] = run
    return run


def _host_prep(pts, means3D, opacities, semantics, scales, cov3D, origin_use):
    pts = np.asarray(pts, np.float32).reshape(N, 3)
    mu32 = np.asarray(means3D, np.float32).reshape(P, 3)
    op = np.asarray(opacities, np.float64).reshape(P)
    sem = np.asarray(semantics, np.float32).reshape(P, C)
    sc32 = np.asarray(scales, np.float32).reshape(P, 3)
    cov = np.asarray(cov3D, np.float64).reshape(P, 3, 3)
    org32 = np.asarray(origin_use, np.float32).reshape(3)

    # --- integer binning, replicated in fp32 exactly like the reference ---
    radii = np.ceil(sc32.max(-1) * np.float32(SCALE_MULT) / np.float32(GRID)
                    ).astype(np.int32).astype(np.int64)
    m_int = ((mu32 - org32) / np.float32(GRID)).astype(np.int32).astype(np.int64)
    p_int = ((pts - org32) / np.float32(GRID)).astype(np.int32).astype(np.int64)

    # structured-input check: points must be the (i, j, k) voxel-center grid
    idx = np.arange(N)
    kk = idx % D
    col = idx // D
    jj = col % W
    ii = col // W
    grid_int = np.stack([ii, jj, kk], axis=-1)
    if not np.array_equal(p_int, grid_int):
        raise RuntimeError("kernel: unstructured pts not supported by fast path")

    # --- per-Gaussian E coefficients (float64 for the inverse, cast to f32) ---
    a, b, c_, d, e, f = (cov[:, 0, 0], cov[:, 1, 1], cov[:, 2, 2],
                         cov[:, 0, 1], cov[:, 1, 2], cov[:, 0, 2])
    det = a * (b * c_ - e * e) - d * (d * c_ - e * f) + f * (d * e - b * f)
    ixx = (b * c_ - e * e) / det
    iyy = (a * c_ - f * f) / det
    izz = (a * b - d * d) / det
    ixy = (e * f - d * c_) / det
    iyz = (d * f - a * e) / det
    ixz = (d * e - b * f) / det
    A = np.empty((P, 3, 3))
    A[:, 0, 0], A[:, 1, 1], A[:, 2, 2] = ixx, iyy, izz
    A[:, 0, 1] = A[:, 1, 0] = ixy
    A[:, 1, 2] = A[:, 2, 1] = iyz
    A[:, 0, 2] = A[:, 2, 0] = ixz
    mu = mu32.astype(np.float64)
    Amu = np.einsum('pij,pj->pi', A, mu)
    muAmu = np.einsum('pi,pi->p', mu, Amu)
    coef10 = np.stack([
        -0.5 * ixx, -0.5 * iyy, -0.5 * izz,
        -ixy, -iyz, -ixz,
        Amu[:, 0], Amu[:, 1], Amu[:, 2],
        -0.5 * muAmu + np.log(op),
    ]).astype(np.float32)                                    # [10, P]

    # --- per-core RHS feature matrices ---
    x, y, z = pts[:, 0], pts[:, 1], pts[:, 2]
    feat10 = np.stack([x * x, y * y, z * z, x * y, y * z, x * z,
                       x, y, z, np.ones_like(x)])            # [10, N] f32
    nloc = np.arange(NPC)
    cg = (nloc % GPTS) // D
    kz = nloc % D
    onehot = np.zeros((GCOLS + D, NPC), np.float32)
    onehot[cg, nloc] = 1.0
    onehot[GCOLS + kz, nloc] = 1.0
    rhs_all = []
    for ci in range(NCORES):
        r = np.empty((KF, NPC), np.float32)
        r[:10] = feat10[:, ci * NPC:(ci + 1) * NPC]
        r[10:] = onehot
        rhs_all.append(r)

    # --- per-(core, group) Gaussian hit lists and penalty tables ---
    mx, my, mz = m_int[:, 0], m_int[:, 1], m_int[:, 2]
    hits_cg = [[None] * NG for _ in range(NCORES)]
    for ci in range(NCORES):
        for g in range(NG):
            col0 = ci * (NPC // D) + g * GCOLS
            cols = col0 + np.arange(GCOLS)
            gi, gj = cols // W, cols % W
            ox = (mx + radii >= gi.min()) & (mx - radii <= gi.max())
            oy = (my + radii >= gj.min()) & (my - radii <= gj.max())
            hits_cg[ci][g] = np.where(ox & oy)[0]
    L_slots = [max(1, max(len(hits_cg[ci][g]) for ci in range(NCORES)))
               for g in range(NG)]
    Ltot = int(np.sum(L_slots))
    offs = np.concatenate([[0], np.cumsum(L_slots)]).astype(int)

    units = []
    for g in range(NG):
        L = int(L_slots[g])
        for s in range(0, L, 128):
            units.append((g, int(offs[g]) + s, min(128, L - s)))
    NU = len(units)

    in_maps = []
    for ci in range(NCORES):
        coef_m = np.zeros((KF, Ltot), np.float32)
        sem_m = np.zeros((Ltot, C), np.float16)
        for g in range(NG):
            hit = hits_cg[ci][g]
            nh = len(hit)
            if nh == 0:
                continue
            o = offs[g]
            coef_m[:10, o:o + nh] = coef10[:, hit]
            col0 = ci * (NPC // D) + g * GCOLS
            cols = col0 + np.arange(GCOLS)
            gi, gj = cols // W, cols % W
            in_xy = ((np.abs(gi[None, :] - mx[hit, None]) <= radii[hit, None]) &
                     (np.abs(gj[None, :] - my[hit, None]) <= radii[hit, None]))
            coef_m[10:10 + GCOLS, o:o + nh] = np.where(in_xy, 0.0, PEN).T
            in_z = (np.abs(np.arange(D)[None, :] - mz[hit, None])
                    <= radii[hit, None])
            coef_m[10 + GCOLS:, o:o + nh] = np.where(in_z, 0.0, PEN).T
            sem_m[o:o + nh] = sem[hit].astype(np.float16)
        semp = np.zeros((128, NU * C), np.float16)
        for u, (g, off, Mt) in enumerate(units):
            semp[0:Mt, u * C:(u + 1) * C] = sem_m[off:off + Mt]
        in_maps.append({"RHS": rhs_all[ci], "COEF": coef_m, "SEMP": semp})
    return in_maps, L_slots


def kernel(**inputs):
    in_maps, L_slots = _host_prep(**inputs)
    nc = _get_nc(L_slots)
    run = _get_runner(nc)
    results, _, _ = run(in_maps)
    out = np.empty((N, C), np.float32)
    for ci in range(NCORES):
        out[ci * NPC:(ci + 1) * NPC] = results[ci]["OUT"].T
    return out



# revision 8
# speedup vs baseline: 2.7202x; 2.7202x over previous
# Trainium2 Bass kernel for nn_LocalAggregator (Gaussian -> voxel-grid semantic
# compositing).
#
# Strategy: the voxel grid (60,60,36) is tiled into 360 3-D blocks of
# (5,6,12) = 360 voxels.  Blocks are dealt to the 8 cores by sorted hit-count
# (rank 8k+c -> core c, slot k) so every core sees a near-identical load
# profile; the host un-permutes the output afterwards.  For each block the
# host builds the exact list of Gaussians whose integer box overlaps the
# block in all three dims.  In block-local coordinates the feature matrix
# (10 quadratic monomials + one-hot rows for the x/y/z box tests) is the SAME
# for every block, so a single [K,360] RHS is shared by all matmuls; all
# per-(block,Gaussian) data lives in the COEF matrix.  The E matmul runs in
# float32r (full-rate on TRN2 for moving dim >= 256; block-centering keeps
# the monomial magnitudes small enough for its relaxed precision), Exp runs
# on the scalar engine over 3 PSUM banks per instruction, and the semantic
# reduction is an fp16 matmul accumulated into a 9-block-packed PSUM bank
# ([117,360]) so one DVE copy + one DMA drains 9 blocks at once.
import numpy as np
import ml_dtypes

H, W, D = 60, 60, 36
GRID = 0.08
SCALE_MULT = 3.0
P = 2048
C = 13
N = H * W * D                  # 129600
NCORES = 8
BX, BY, BZ = 5, 6, 12          # block shape
NBX, NBY, NBZ = H // BX, W // BY, D // BZ
NB = NBX * NBY * NBZ           # 360 blocks total
GPTS = BX * BY * BZ            # 360 points per block
NSLOT = NB // NCORES           # 45 blocks (slots) per core
NPC = NSLOT * GPTS             # 16200 points per core
KF = 20 + BX + BY + BZ         # 43 feature rows (10 monomials x hi/lo split)
PEN = -2000.0                  # box-miss penalty (exp() == 0 in fp32)
OGRP = 3                       # blocks packed per PSUM output bank (stride 32)
OSTR = 32                      # partition stride between packed blocks
OROWS = (OGRP - 1) * OSTR + C  # 77 live partitions per output group
NGRP = NSLOT // OGRP           # 15 output groups per core
ACT3 = 3                       # units per Exp instruction / psE tile

_NC_CACHE: dict = {}
_JIT_CACHE: dict = {}


def _build_nc(L_slots):
    import concourse.bacc as bacc
    import concourse.tile as tile
    from concourse import mybir

    L_slots = [int(x) for x in L_slots]
    offs = np.concatenate([[0], np.cumsum(L_slots)]).astype(int)
    Ltot = int(offs[-1])
    # units: (slot, coef col offset, Mt, first-in-slot, last-in-slot)
    units = []
    for g in range(NSLOT):
        L = L_slots[g]
        for s in range(0, L, 128):
            units.append((g, int(offs[g]) + s, min(128, L - s),
                          s == 0, s + 128 >= L))
    NU = len(units)
    triples = [units[i:i + ACT3] for i in range(0, NU, ACT3)]

    nc = bacc.Bacc("TRN2", target_bir_lowering=False, debug=False,
                   num_devices=NCORES)
    f32 = mybir.dt.float32
    fmm = mybir.dt.float32r
    f16 = mybir.dt.float16
    RHS = nc.dram_tensor("RHS", [KF, GPTS], fmm, kind="ExternalInput")
    COEF = nc.dram_tensor("COEF", [KF, Ltot], fmm, kind="ExternalInput")
    SEMP = nc.dram_tensor("SEMP", [128, NU * C], f16, kind="ExternalInput")
    OUT = nc.dram_tensor("OUT", [OROWS, NGRP * GPTS], f16,
                         kind="ExternalOutput")

    with tile.TileContext(nc) as tc:
        with (
            tc.tile_pool(name="big", bufs=1) as big_pool,
            tc.tile_pool(name="w", bufs=4) as w_pool,
            tc.tile_pool(name="og", bufs=3) as og_pool,
            tc.tile_pool(name="psE", bufs=2, space="PSUM") as pse_pool,
            tc.tile_pool(name="psO", bufs=2, space="PSUM") as pso_pool,
        ):
            rhs_b = big_pool.tile([KF, GPTS], fmm)
            coef_b = big_pool.tile([KF, Ltot], fmm)
            semp_b = big_pool.tile([128, NU * C], f16)
            nc.sync.dma_start(rhs_b[:], RHS[:])
            # chunked loads so compute can start after the first slice lands
            NCH = 5
            for ch in range(NCH):
                g0, g1 = ch * NSLOT // NCH, (ch + 1) * NSLOT // NCH
                a, b = int(offs[g0]), int(offs[g1])
                nc.sync.dma_start(coef_b[:, a:b], COEF[:, a:b])
                u0 = sum(1 for u in units if u[0] < g0)
                u1 = sum(1 for u in units if u[0] < g1)
                nc.sync.dma_start(semp_b[:, u0 * C:u1 * C],
                                  SEMP[:, u0 * C:u1 * C])

            pso_t = {}    # group -> psO tile
            uidx = 0
            for tri in triples:
                psE = pse_pool.tile([128, ACT3, 512], f32)
                w_t = w_pool.tile([128, ACT3, GPTS], f16)
                mtmax = max(u[2] for u in tri)
                for j, (g, off, Mt, first, last) in enumerate(tri):
                    nc.tensor.matmul(psE[0:Mt, j:j + 1, 0:GPTS],
                                     coef_b[:, off:off + Mt],
                                     rhs_b[:],
                                     start=True, stop=True)
                nc.scalar.activation(w_t[0:mtmax, 0:len(tri), :],
                                     psE[0:mtmax, 0:len(tri), 0:GPTS],
                                     mybir.ActivationFunctionType.Exp)
                for j, (g, off, Mt, first, last) in enumerate(tri):
                    grp, gi = g // OGRP, g % OGRP
                    if first and gi == 0:
                        pso_t[grp] = pso_pool.tile([OROWS, GPTS], f32,
                                                   name=f"psO{grp}",
                                                   tag="psO")
                    nc.tensor.matmul(
                        pso_t[grp][gi * OSTR:gi * OSTR + C, :],
                        semp_b[0:Mt, (uidx + j) * C:(uidx + j + 1) * C],
                        w_t[0:Mt, j:j + 1, :],
                        start=first, stop=last, skip_group_check=True)
                    if last and gi == OGRP - 1:
                        outg = og_pool.tile([OROWS, GPTS], f16)
                        nc.vector.tensor_copy(outg[:], pso_t[grp][:])
                        nc.sync.dma_start(
                            OUT[:, grp * GPTS:(grp + 1) * GPTS], outg[:])
                uidx += len(tri)
    nc.compile()
    return nc


def _get_nc(L_slots):
    key = tuple(int(x) for x in L_slots)
    if key not in _NC_CACHE:
        _NC_CACHE[key] = _build_nc(L_slots)
    return _NC_CACHE[key]


def _get_runner(nc):
    """Cached shard_map-jitted executor for one Bass program (axon/PJRT path).

    Mirrors concourse.bass2jax.run_bass_via_pjrt but keeps the jitted callable
    so repeated runs don't rebuild/recompile."""
    if id(nc) in _JIT_CACHE:
        return _JIT_CACHE[id(nc)]
    import jax
    from concourse import bass2jax, mybir
    from jax.experimental.shard_map import shard_map
    from jax.sharding import Mesh, PartitionSpec

    bass2jax.install_neuronx_cc_hook()
    partition_name = (nc.partition_id_tensor.name
                      if nc.partition_id_tensor else None)
    in_names, out_names, out_avals, zero_outs = [], [], [], []
    for alloc in nc.m.functions[0].allocations:
        if not isinstance(alloc, mybir.MemoryLocationSet):
            continue
        name = alloc.memorylocations[0].name
        if alloc.kind == "ExternalInput":
            if name == partition_name:
                continue
            in_names.append(name)
        elif alloc.kind == "ExternalOutput":
            shape = tuple(alloc.tensor_shape)
            dtype = mybir.dt.np(alloc.dtype)
            out_names.append(name)
            out_avals.append(jax.core.ShapedArray(shape, dtype))
            zero_outs.append(np.zeros(shape, dtype))
    n_params = len(in_names)
    all_in_names = in_names + out_names
    if partition_name is not None:
        all_in_names = all_in_names + [partition_name]

    def _body(*args):
        operands = list(args)
        if partition_name is not None:
            operands.append(bass2jax.partition_id_tensor())
        outs = bass2jax._bass_exec_p.bind(
            *operands,
            out_avals=tuple(out_avals),
            in_names=tuple(all_in_names),
            out_names=tuple(out_names),
            lowering_input_output_aliases=(),
            sim_require_finite=True,
            sim_require_nnan=True,
            nc=nc,
        )
        return tuple(outs)

    devices = jax.devices()[:NCORES]
    mesh = Mesh(np.asarray(devices), ("core",))
    donate = tuple(range(n_params, n_params + len(out_names)))
    sharded = jax.jit(
        shard_map(_body, mesh=mesh,
                  in_specs=(PartitionSpec("core"),) * (n_params + len(out_names)),
                  out_specs=(PartitionSpec("core"),) * len(out_names),
                  check_rep=False),
        donate_argnums=donate, keep_unused=True)

    def run(in_maps, rounds=1):
        concat_in = [np.concatenate([np.asarray(m[nm]) for m in in_maps], axis=0)
                     for nm in in_names]
        outs = None
        for _ in range(rounds):
            zo = [np.concatenate([z] * NCORES, axis=0) for z in zero_outs]
            outs = sharded(*concat_in, *zo)
        outs = [np.asarray(o) for o in outs]
        results = []
        for ci in range(NCORES):
            d = {}
            for oi, nm in enumerate(out_names):
                per = outs[oi].shape[0] // NCORES
                d[nm] = outs[oi][ci * per:(ci + 1) * per]
            results.append(d)
        return results, sharded, (concat_in, zero_outs, in_names, out_names)

    sharded_nd = jax.jit(
        shard_map(_body, mesh=mesh,
                  in_specs=(PartitionSpec("core"),) * (n_params + len(out_names)),
                  out_specs=(PartitionSpec("core"),) * len(out_names),
                  check_rep=False),
        keep_unused=True)

    def timeit(in_maps, iters=30):
        import time as _time
        from jax.sharding import NamedSharding
        sh = NamedSharding(mesh, PartitionSpec("core"))
        concat_in = [np.concatenate([np.asarray(m[nm]) for m in in_maps], axis=0)
                     for nm in in_names]
        zo = [np.concatenate([z] * NCORES, axis=0) for z in zero_outs]
        args = [jax.device_put(a, sh) for a in concat_in + zo]
        outs = sharded_nd(*args)
        jax.block_until_ready(outs)
        t0 = _time.time()
        for _ in range(iters):
            outs = sharded_nd(*args)
        jax.block_until_ready(outs)
        return (_time.time() - t0) / iters

    run.timeit = timeit
    _JIT_CACHE[id(nc)] = run
    return run


def _host_prep(pts, means3D, opacities, semantics, scales, cov3D, origin_use):
    pts = np.asarray(pts, np.float32).reshape(N, 3)
    mu32 = np.asarray(means3D, np.float32).reshape(P, 3)
    op = np.asarray(opacities, np.float64).reshape(P)
    sem = np.asarray(semantics, np.float32).reshape(P, C)
    sc32 = np.asarray(scales, np.float32).reshape(P, 3)
    cov = np.asarray(cov3D, np.float64).reshape(P, 3, 3)
    org32 = np.asarray(origin_use, np.float32).reshape(3)

    # --- integer binning, replicated in fp32 exactly like the reference ---
    radii = np.ceil(sc32.max(-1) * np.float32(SCALE_MULT) / np.float32(GRID)
                    ).astype(np.int32).astype(np.int64)
    m_int = ((mu32 - org32) / np.float32(GRID)).astype(np.int32).astype(np.int64)
    p_int = ((pts - org32) / np.float32(GRID)).astype(np.int32).astype(np.int64)

    # structured-input check: points must be the (i, j, k) voxel-center grid
    idx = np.arange(N)
    kk = idx % D
    col = idx // D
    jj = col % W
    ii = col // W
    grid_int = np.stack([ii, jj, kk], axis=-1)
    if not np.array_equal(p_int, grid_int):
        raise RuntimeError("kernel: unstructured pts not supported by fast path")

    # --- per-Gaussian inverse covariance (float64) ---
    a, b, c_, d, e, f = (cov[:, 0, 0], cov[:, 1, 1], cov[:, 2, 2],
                         cov[:, 0, 1], cov[:, 1, 2], cov[:, 0, 2])
    det = a * (b * c_ - e * e) - d * (d * c_ - e * f) + f * (d * e - b * f)
    ixx = (b * c_ - e * e) / det
    iyy = (a * c_ - f * f) / det
    izz = (a * b - d * d) / det
    ixy = (e * f - d * c_) / det
    iyz = (d * f - a * e) / det
    ixz = (d * e - b * f) / det
    A = np.empty((P, 3, 3))
    A[:, 0, 0], A[:, 1, 1], A[:, 2, 2] = ixx, iyy, izz
    A[:, 0, 1] = A[:, 1, 0] = ixy
    A[:, 1, 2] = A[:, 2, 1] = iyz
    A[:, 0, 2] = A[:, 2, 0] = ixz
    mu = mu32.astype(np.float64)
    logop = np.log(op)

    # --- blocks: bounds, hit lists, load-balanced assignment ---
    mx, my, mz = m_int[:, 0], m_int[:, 1], m_int[:, 2]
    blocks = []            # (x0, y0, z0)
    hits = []
    for bxi in range(NBX):
        for byi in range(NBY):
            for bzi in range(NBZ):
                x0, y0, z0 = bxi * BX, byi * BY, bzi * BZ
                hit = np.where(
                    (mx + radii >= x0) & (mx - radii <= x0 + BX - 1) &
                    (my + radii >= y0) & (my - radii <= y0 + BY - 1) &
                    (mz + radii >= z0) & (mz - radii <= z0 + BZ - 1))[0]
                blocks.append((x0, y0, z0))
                hits.append(hit)
    Ls = np.array([len(h) for h in hits])
    order = np.argsort(-Ls, kind="stable")     # rank r -> block id
    # core c, slot k gets block order[8k + c]; slot size = L of rank 8k
    L_slots = [max(1, int(Ls[order[8 * k]])) for k in range(NSLOT)]
    offs = np.concatenate([[0], np.cumsum(L_slots)]).astype(int)
    Ltot = int(offs[-1])
    units = []
    for g in range(NSLOT):
        L = L_slots[g]
        for s in range(0, L, 128):
            units.append((g, int(offs[g]) + s, min(128, L - s)))
    NU = len(units)

    # --- shared RHS: block-local features + one-hot rows  [KF, GPTS] ---
    lx = np.arange(GPTS) // (BY * BZ)
    ly = (np.arange(GPTS) // BZ) % BY
    lz = np.arange(GPTS) % BZ
    xi = (lx - (BX - 1) / 2.0) * GRID
    yi = (ly - (BY - 1) / 2.0) * GRID
    zi = (lz - (BZ - 1) / 2.0) * GRID
    rhs = np.zeros((KF, GPTS), np.float32)
    feat10 = np.stack([xi * xi, yi * yi, zi * zi, xi * yi, yi * zi, xi * zi,
                       xi, yi, zi, np.ones(GPTS)]).astype(np.float32)
    rhs[0:10] = feat10
    rhs[10:20] = feat10          # same features for the lo residual rows
    rhs[20 + lx, np.arange(GPTS)] = 1.0
    rhs[20 + BX + ly, np.arange(GPTS)] = 1.0
    rhs[20 + BX + BY + lz, np.arange(GPTS)] = 1.0

    # --- per-core COEF / SEMP ---
    in_maps = []
    perm_blocks = []       # per core: slot -> block id
    for ci in range(NCORES):
        coef_m = np.zeros((KF, Ltot), np.float32)
        sem_m = np.zeros((Ltot, C), np.float16)
        my_blocks = []
        for g in range(NSLOT):
            bid = int(order[8 * g + ci])
            my_blocks.append(bid)
            hit = hits[bid]
            nh = len(hit)
            if nh == 0:
                continue
            o = offs[g]
            x0, y0, z0 = blocks[bid]
            # block center in f64
            cx = (x0 + (BX - 1) / 2.0 + 0.5) * GRID
            cy = (y0 + (BY - 1) / 2.0 + 0.5) * GRID
            cz = (z0 + (BZ - 1) / 2.0 + 0.5) * GRID
            dmu = mu[hit] - np.array([cx, cy, cz])      # [nh, 3]
            Ah = A[hit]                                  # [nh, 3, 3]
            Amu = np.einsum('pij,pj->pi', Ah, dmu)
            muAmu = np.einsum('pi,pi->p', dmu, Amu)
            c10 = np.stack([
                -0.5 * Ah[:, 0, 0], -0.5 * Ah[:, 1, 1], -0.5 * Ah[:, 2, 2],
                -Ah[:, 0, 1], -Ah[:, 1, 2], -Ah[:, 0, 2],
                Amu[:, 0], Amu[:, 1], Amu[:, 2],
                -0.5 * muAmu + logop[hit]]).astype(np.float32)
            hi = c10.astype(ml_dtypes.bfloat16).astype(np.float32)
            coef_m[0:10, o:o + nh] = hi
            coef_m[10:20, o:o + nh] = c10 - hi
            # box penalties per dim (0 if inside, PEN outside)
            in_x = (np.abs(x0 + np.arange(BX)[None, :] - mx[hit, None])
                    <= radii[hit, None])
            in_y = (np.abs(y0 + np.arange(BY)[None, :] - my[hit, None])
                    <= radii[hit, None])
            in_z = (np.abs(z0 + np.arange(BZ)[None, :] - mz[hit, None])
                    <= radii[hit, None])
            coef_m[20:20 + BX, o:o + nh] = np.where(in_x, 0.0, PEN).T
            coef_m[20 + BX:20 + BX + BY, o:o + nh] = np.where(in_y, 0.0, PEN).T
            coef_m[20 + BX + BY:, o:o + nh] = np.where(in_z, 0.0, PEN).T
            sem_m[o:o + nh] = sem[hit].astype(np.float16)
        semp = np.zeros((128, NU * C), np.float16)
        for u, (g, off, Mt) in enumerate(units):
            semp[0:Mt, u * C:(u + 1) * C] = sem_m[off:off + Mt]
        perm_blocks.append(my_blocks)
        in_maps.append({"RHS": rhs, "COEF": coef_m, "SEMP": semp})
    return in_maps, L_slots, perm_blocks


def kernel(**inputs):
    in_maps, L_slots, perm_blocks = _host_prep(**inputs)
    nc = _get_nc(L_slots)
    run = _get_runner(nc)
    results, _, _ = run(in_maps)
    out = np.empty((N, C), np.float32)
    lx = np.arange(GPTS) // (BY * BZ)
    ly = (np.arange(GPTS) // BZ) % BY
    lz = np.arange(GPTS) % BZ
    for ci in range(NCORES):
        o = results[ci]["OUT"].astype(np.float32)   # [OROWS, NGRP*GPTS]
        for g in range(NSLOT):
            bid = perm_blocks[ci][g]
            x0, y0, z0 = (bid // (NBY * NBZ)) * BX, \
                         ((bid // NBZ) % NBY) * BY, (bid % NBZ) * BZ
            grp, gi = g // OGRP, g % OGRP
            blk = o[gi * OSTR:gi * OSTR + C,
                    grp * GPTS:(grp + 1) * GPTS]               # [C,GPTS]
            gidx = ((x0 + lx) * W + (y0 + ly)) * D + (z0 + lz)
            out[gidx] = blk.T
    return out


# revision 11
# speedup vs baseline: 3.0539x; 1.1227x over previous
# Trainium2 Bass kernel for nn_LocalAggregator (Gaussian -> voxel-grid semantic
# compositing).
#
# Strategy: the voxel grid (60,60,36) is tiled into 1080 3-D blocks of
# (4,5,6) = 120 voxels.  Blocks are dealt to the 8 cores by sorted hit-count
# (rank 8k+c -> core c, slot k) so every core sees a near-identical load
# profile; the host un-permutes the output afterwards.  For each block the
# host builds the exact list of Gaussians whose integer box overlaps the
# block in all three dims.  In block-local coordinates the feature matrix
# (quadratic monomials + one-hot rows for the x/y/z box tests) is the SAME
# for every block, so a single small RHS is shared by all matmuls; all
# per-(block,Gaussian) data lives in the COEF matrix.  The E matmul runs in
# bf16 with a 3-way hi/lo product split (rows [hi,hi,lo] x features
# [fhi,flo,fhi] drop only the lo*lo term, ~1e-5 relative), Exp runs on the
# scalar engine over 12 units (3 PSUM banks x 4 blocks each) at a time, and
# the semantic reduction is an fp16 matmul accumulated into PSUM banks packed
# 12 blocks deep (3 partition stripes x 4 column slots) so one DVE copy +
# one Pool-engine DMA drains 12 blocks at once.  Dummy matmuls at t=0 ramp
# the PE clock to max p-state and a dummy activation preloads the Exp table.
import numpy as np
import ml_dtypes

H, W, D = 60, 60, 36
GRID = 0.08
SCALE_MULT = 3.0
P = 2048
C = 13
N = H * W * D                  # 129600
NCORES = 8
BX, BY, BZ = 4, 5, 6           # block shape
NBX, NBY, NBZ = H // BX, W // BY, D // BZ
NB = NBX * NBY * NBZ           # 1080 blocks total
GPTS = BX * BY * BZ            # 120 points per block
NSLOT = NB // NCORES           # 135 blocks (slots) per core
NPC = NSLOT * GPTS             # 16200 points per core
KF = 30 + BX + BY + BZ         # 45 feature rows (3x10 split products + onehots)
PEN = -2000.0                  # box-miss penalty (exp() == 0 in fp32)
PERBANK = 4                    # 120-col units per 512-f32 PSUM bank
NBANK = 3                      # banks per psE tile / act instruction
UPT = PERBANK * NBANK          # 12 units per psE tile
OGRP = 12                      # blocks per psO bank (3 stripes x 4 col slots)
OROWS = 77                     # 2*32+13 live partitions per output group
OCOLS = PERBANK * GPTS         # 480 cols per output group
NGRP = (NSLOT + OGRP - 1) // OGRP   # 12 output groups per core (last partial)

_NC_CACHE: dict = {}
_JIT_CACHE: dict = {}


def _build_nc(L_slots):
    import concourse.bacc as bacc
    import concourse.tile as tile
    from concourse import mybir

    L_slots = [int(x) for x in L_slots]
    offs = np.concatenate([[0], np.cumsum(L_slots)]).astype(int)
    Ltot = int(offs[-1])
    # units: (slot, coef col offset, Mt, first-in-slot, last-in-slot)
    units = []
    for g in range(NSLOT):
        L = L_slots[g]
        for s in range(0, L, 128):
            units.append((g, int(offs[g]) + s, min(128, L - s),
                          s == 0, s + 128 >= L))
    NU = len(units)
    tiles_u = [units[i:i + UPT] for i in range(0, NU, UPT)]

    nc = bacc.Bacc("TRN2", target_bir_lowering=False, debug=False,
                   num_devices=NCORES)
    f32 = mybir.dt.float32
    bf16 = mybir.dt.bfloat16
    f16 = mybir.dt.float16
    RHS = nc.dram_tensor("RHS", [KF, GPTS], bf16, kind="ExternalInput")
    COEF = nc.dram_tensor("COEF", [KF, Ltot], bf16, kind="ExternalInput")
    SEMP = nc.dram_tensor("SEMP", [128, NU * C], f16, kind="ExternalInput")
    OUT = nc.dram_tensor("OUT", [OROWS, NGRP * OCOLS], f16,
                         kind="ExternalOutput")

    # coef-chunk boundaries (slots): small first chunk so compute starts early
    CH_SLOTS = [0, 14, 55, NSLOT]
    # semp split: first piece covers the first ~2 chunks of units
    u_mid = sum(1 for u in units if u[0] < CH_SLOTS[1])

    with tile.TileContext(nc) as tc:
        with (
            tc.tile_pool(name="big", bufs=1) as big_pool,
            tc.tile_pool(name="w", bufs=4) as w_pool,
            tc.tile_pool(name="og", bufs=3) as og_pool,
            tc.tile_pool(name="psE", bufs=2, space="PSUM") as pse_pool,
            tc.tile_pool(name="psO", bufs=2, space="PSUM") as pso_pool,
        ):
            rhs_b = big_pool.tile([KF, GPTS], bf16)
            coef_b = big_pool.tile([KF, Ltot], bf16)
            semp_b = big_pool.tile([128, NU * C], f16)
            scr_b = big_pool.tile([1, 512], bf16)
            scr_o = big_pool.tile([1, 8], f16)

            # --- warmup: PE p-state ramp + activation table preload ---
            nc.gpsimd.memset(scr_b[:], 0.0)
            nc.scalar.activation(scr_o[0:1, 0:1], scr_b[0:1, 0:1],
                                 mybir.ActivationFunctionType.Exp)
            psD = pse_pool.tile([128, NBANK, 512], f32, name="psD", tag="psE")
            for _ in range(3):
                nc.tensor.matmul(psD[0:1, 0:1, 0:512], scr_b[0:1, 0:1],
                                 scr_b[0:1, 0:512], start=True, stop=True,
                                 skip_group_check=True)

            # --- input loads ---
            # RHS via Pool (SWDGE), COEF/SEMP chunks via SP (HWDGE), ordered
            # so the first compute chunk lands earliest.
            nc.gpsimd.dma_start(rhs_b[:], RHS[:])
            a0, a1 = int(offs[CH_SLOTS[0]]), int(offs[CH_SLOTS[1]])
            nc.sync.dma_start(coef_b[:, a0:a1], COEF[:, a0:a1])
            nc.sync.dma_start(semp_b[:, 0:u_mid * C], SEMP[:, 0:u_mid * C])
            a1b, a2 = int(offs[CH_SLOTS[1]]), int(offs[CH_SLOTS[2]])
            nc.sync.dma_start(coef_b[:, a1b:a2], COEF[:, a1b:a2])
            nc.sync.dma_start(semp_b[:, u_mid * C:], SEMP[:, u_mid * C:])
            a2b, a3 = int(offs[CH_SLOTS[2]]), int(offs[CH_SLOTS[3]])
            nc.sync.dma_start(coef_b[:, a2b:a3], COEF[:, a2b:a3])

            pso_t = {}    # group -> psO tile
            uidx = 0
            for tu in tiles_u:
                psE = pse_pool.tile([128, NBANK, 512], f32, tag="psE")
                w_t = w_pool.tile([128, NBANK, OCOLS], f16)
                mtmax = max(u[2] for u in tu)
                for j, (g, off, Mt, first, last) in enumerate(tu):
                    b, s = j // PERBANK, j % PERBANK
                    nc.tensor.matmul(
                        psE[0:Mt, b:b + 1, s * GPTS:(s + 1) * GPTS],
                        coef_b[:, off:off + Mt], rhs_b[:],
                        start=True, stop=True, skip_group_check=True)
                nb_full, rem = divmod(len(tu), PERBANK)
                if nb_full:
                    nc.scalar.activation(
                        w_t[0:mtmax, 0:nb_full, :],
                        psE[0:mtmax, 0:nb_full, 0:OCOLS],
                        mybir.ActivationFunctionType.Exp)
                if rem:
                    nc.scalar.activation(
                        w_t[0:mtmax, nb_full:nb_full + 1, 0:rem * GPTS],
                        psE[0:mtmax, nb_full:nb_full + 1, 0:rem * GPTS],
                        mybir.ActivationFunctionType.Exp)
                for j, (g, off, Mt, first, last) in enumerate(tu):
                    grp, gg = g // OGRP, g % OGRP
                    gi, s = gg // PERBANK, gg % PERBANK
                    if first and gg == 0:
                        pso_t[grp] = pso_pool.tile([OROWS, OCOLS], f32,
                                                   name=f"psO{grp}",
                                                   tag="psO")
                    b, js = j // PERBANK, j % PERBANK
                    nc.tensor.matmul(
                        pso_t[grp][gi * 32:gi * 32 + C,
                                   s * GPTS:(s + 1) * GPTS],
                        semp_b[0:Mt, (uidx + j) * C:(uidx + j + 1) * C],
                        w_t[0:Mt, b:b + 1, js * GPTS:(js + 1) * GPTS],
                        start=first, stop=last, skip_group_check=True)
                    if last and (g == NSLOT - 1 or gg == OGRP - 1):
                        nblk = g % OGRP + 1
                        top = gg // PERBANK            # last stripe index
                        grows = top * 32 + C
                        gcols = OCOLS if top > 0 else nblk * GPTS
                        outg = og_pool.tile([OROWS, OCOLS], f16)
                        nc.vector.tensor_copy(outg[0:grows, 0:gcols],
                                              pso_t[grp][0:grows, 0:gcols])
                        nc.gpsimd.dma_start(
                            OUT[0:grows, grp * OCOLS:grp * OCOLS + gcols],
                            outg[0:grows, 0:gcols])
                uidx += len(tu)
    nc.compile()
    return nc


def _get_nc(L_slots):
    key = tuple(int(x) for x in L_slots)
    if key not in _NC_CACHE:
        _NC_CACHE[key] = _build_nc(L_slots)
    return _NC_CACHE[key]


def _get_runner(nc):
    """Cached shard_map-jitted executor for one Bass program (axon/PJRT path).

    Mirrors concourse.bass2jax.run_bass_via_pjrt but keeps the jitted callable
    so repeated runs don't rebuild/recompile."""
    if id(nc) in _JIT_CACHE:
        return _JIT_CACHE[id(nc)]
    import jax
    from concourse import bass2jax, mybir
    from jax.experimental.shard_map import shard_map
    from jax.sharding import Mesh, PartitionSpec

    bass2jax.install_neuronx_cc_hook()
    partition_name = (nc.partition_id_tensor.name
                      if nc.partition_id_tensor else None)
    in_names, out_names, out_avals, zero_outs = [], [], [], []
    for alloc in nc.m.functions[0].allocations:
        if not isinstance(alloc, mybir.MemoryLocationSet):
            continue
        name = alloc.memorylocations[0].name
        if alloc.kind == "ExternalInput":
            if name == partition_name:
                continue
            in_names.append(name)
        elif alloc.kind == "ExternalOutput":
            shape = tuple(alloc.tensor_shape)
            dtype = mybir.dt.np(alloc.dtype)
            out_names.append(name)
            out_avals.append(jax.core.ShapedArray(shape, dtype))
            zero_outs.append(np.zeros(shape, dtype))
    n_params = len(in_names)
    all_in_names = in_names + out_names
    if partition_name is not None:
        all_in_names = all_in_names + [partition_name]

    def _body(*args):
        operands = list(args)
        if partition_name is not None:
            operands.append(bass2jax.partition_id_tensor())
        outs = bass2jax._bass_exec_p.bind(
            *operands,
            out_avals=tuple(out_avals),
            in_names=tuple(all_in_names),
            out_names=tuple(out_names),
            lowering_input_output_aliases=(),
            sim_require_finite=True,
            sim_require_nnan=True,
            nc=nc,
        )
        return tuple(outs)

    devices = jax.devices()[:NCORES]
    mesh = Mesh(np.asarray(devices), ("core",))
    donate = tuple(range(n_params, n_params + len(out_names)))
    sharded = jax.jit(
        shard_map(_body, mesh=mesh,
                  in_specs=(PartitionSpec("core"),) * (n_params + len(out_names)),
                  out_specs=(PartitionSpec("core"),) * len(out_names),
                  check_rep=False),
        donate_argnums=donate, keep_unused=True)

    def run(in_maps, rounds=1):
        concat_in = [np.concatenate([np.asarray(m[nm]) for m in in_maps], axis=0)
                     for nm in in_names]
        outs = None
        for _ in range(rounds):
            zo = [np.concatenate([z] * NCORES, axis=0) for z in zero_outs]
            outs = sharded(*concat_in, *zo)
        outs = [np.asarray(o) for o in outs]
        results = []
        for ci in range(NCORES):
            d = {}
            for oi, nm in enumerate(out_names):
                per = outs[oi].shape[0] // NCORES
                d[nm] = outs[oi][ci * per:(ci + 1) * per]
            results.append(d)
        return results, sharded, (concat_in, zero_outs, in_names, out_names)

    sharded_nd = jax.jit(
        shard_map(_body, mesh=mesh,
                  in_specs=(PartitionSpec("core"),) * (n_params + len(out_names)),
                  out_specs=(PartitionSpec("core"),) * len(out_names),
                  check_rep=False),
        keep_unused=True)

    def timeit(in_maps, iters=30):
        import time as _time
        from jax.sharding import NamedSharding
        sh = NamedSharding(mesh, PartitionSpec("core"))
        concat_in = [np.concatenate([np.asarray(m[nm]) for m in in_maps], axis=0)
                     for nm in in_names]
        zo = [np.concatenate([z] * NCORES, axis=0) for z in zero_outs]
        args = [jax.device_put(a, sh) for a in concat_in + zo]
        outs = sharded_nd(*args)
        jax.block_until_ready(outs)
        t0 = _time.time()
        for _ in range(iters):
            outs = sharded_nd(*args)
        jax.block_until_ready(outs)
        return (_time.time() - t0) / iters

    run.timeit = timeit
    _JIT_CACHE[id(nc)] = run
    return run


def _bf16_split(x):
    hi = x.astype(ml_dtypes.bfloat16).astype(np.float32)
    return hi, (x - hi).astype(np.float32)


def _host_prep(pts, means3D, opacities, semantics, scales, cov3D, origin_use):
    pts = np.asarray(pts, np.float32).reshape(N, 3)
    mu32 = np.asarray(means3D, np.float32).reshape(P, 3)
    op = np.asarray(opacities, np.float64).reshape(P)
    sem = np.asarray(semantics, np.float32).reshape(P, C)
    sc32 = np.asarray(scales, np.float32).reshape(P, 3)
    cov = np.asarray(cov3D, np.float64).reshape(P, 3, 3)
    org32 = np.asarray(origin_use, np.float32).reshape(3)

    # --- integer binning, replicated in fp32 exactly like the reference ---
    radii = np.ceil(sc32.max(-1) * np.float32(SCALE_MULT) / np.float32(GRID)
                    ).astype(np.int32).astype(np.int64)
    m_int = ((mu32 - org32) / np.float32(GRID)).astype(np.int32).astype(np.int64)
    p_int = ((pts - org32) / np.float32(GRID)).astype(np.int32).astype(np.int64)

    # structured-input check: points must be the (i, j, k) voxel-center grid
    idx = np.arange(N)
    kk = idx % D
    col = idx // D
    jj = col % W
    ii = col // W
    grid_int = np.stack([ii, jj, kk], axis=-1)
    if not np.array_equal(p_int, grid_int):
        raise RuntimeError("kernel: unstructured pts not supported by fast path")

    # --- per-Gaussian inverse covariance (float64) ---
    a, b, c_, d, e, f = (cov[:, 0, 0], cov[:, 1, 1], cov[:, 2, 2],
                         cov[:, 0, 1], cov[:, 1, 2], cov[:, 0, 2])
    det = a * (b * c_ - e * e) - d * (d * c_ - e * f) + f * (d * e - b * f)
    ixx = (b * c_ - e * e) / det
    iyy = (a * c_ - f * f) / det
    izz = (a * b - d * d) / det
    ixy = (e * f - d * c_) / det
    iyz = (d * f - a * e) / det
    ixz = (d * e - b * f) / det
    A = np.empty((P, 3, 3))
    A[:, 0, 0], A[:, 1, 1], A[:, 2, 2] = ixx, iyy, izz
    A[:, 0, 1] = A[:, 1, 0] = ixy
    A[:, 1, 2] = A[:, 2, 1] = iyz
    A[:, 0, 2] = A[:, 2, 0] = ixz
    mu = mu32.astype(np.float64)
    logop = np.log(op)

    # --- blocks: bounds, hit lists, load-balanced assignment ---
    mx, my, mz = m_int[:, 0], m_int[:, 1], m_int[:, 2]
    blocks = []            # (x0, y0, z0)
    hits = []
    for bxi in range(NBX):
        for byi in range(NBY):
            for bzi in range(NBZ):
                x0, y0, z0 = bxi * BX, byi * BY, bzi * BZ
                hit = np.where(
                    (mx + radii >= x0) & (mx - radii <= x0 + BX - 1) &
                    (my + radii >= y0) & (my - radii <= y0 + BY - 1) &
                    (mz + radii >= z0) & (mz - radii <= z0 + BZ - 1))[0]
                blocks.append((x0, y0, z0))
                hits.append(hit)
    Ls = np.array([len(h) for h in hits])
    order = np.argsort(-Ls, kind="stable")     # rank r -> block id
    # core c, slot k gets block order[8k + c]; slot size = L of rank 8k
    L_slots = [max(1, int(Ls[order[8 * k]])) for k in range(NSLOT)]
    offs = np.concatenate([[0], np.cumsum(L_slots)]).astype(int)
    Ltot = int(offs[-1])
    units = []
    for g in range(NSLOT):
        L = L_slots[g]
        for s in range(0, L, 128):
            units.append((g, int(offs[g]) + s, min(128, L - s)))
    NU = len(units)

    # --- shared RHS: block-local features (hi/lo pairs) + one-hot rows ---
    lx = np.arange(GPTS) // (BY * BZ)
    ly = (np.arange(GPTS) // BZ) % BY
    lz = np.arange(GPTS) % BZ
    xi = ((lx - (BX - 1) / 2.0) * GRID).astype(np.float32)
    yi = ((ly - (BY - 1) / 2.0) * GRID).astype(np.float32)
    zi = ((lz - (BZ - 1) / 2.0) * GRID).astype(np.float32)
    feat10 = np.stack([xi * xi, yi * yi, zi * zi, xi * yi, yi * zi, xi * zi,
                       xi, yi, zi, np.ones(GPTS, np.float32)])
    fhi, flo = _bf16_split(feat10)
    rhs = np.zeros((KF, GPTS), np.float32)
    rhs[0:10] = fhi
    rhs[10:20] = flo
    rhs[20:30] = fhi
    rhs[30 + lx, np.arange(GPTS)] = 1.0
    rhs[30 + BX + ly, np.arange(GPTS)] = 1.0
    rhs[30 + BX + BY + lz, np.arange(GPTS)] = 1.0
    rhs = rhs.astype(ml_dtypes.bfloat16)

    # --- per-core COEF / SEMP ---
    in_maps = []
    perm_blocks = []       # per core: slot -> block id
    for ci in range(NCORES):
        coef_m = np.zeros((KF, Ltot), np.float32)
        sem_m = np.zeros((Ltot, C), np.float16)
        my_blocks = []
        for g in range(NSLOT):
            bid = int(order[8 * g + ci])
            my_blocks.append(bid)
            hit = hits[bid]
            nh = len(hit)
            if nh == 0:
                continue
            o = offs[g]
            x0, y0, z0 = blocks[bid]
            cx = (x0 + (BX - 1) / 2.0 + 0.5) * GRID
            cy = (y0 + (BY - 1) / 2.0 + 0.5) * GRID
            cz = (z0 + (BZ - 1) / 2.0 + 0.5) * GRID
            dmu = mu[hit] - np.array([cx, cy, cz])      # [nh, 3]
            Ah = A[hit]                                  # [nh, 3, 3]
            Amu = np.einsum('pij,pj->pi', Ah, dmu)
            muAmu = np.einsum('pi,pi->p', dmu, Amu)
            c10 = np.stack([
                -0.5 * Ah[:, 0, 0], -0.5 * Ah[:, 1, 1], -0.5 * Ah[:, 2, 2],
                -Ah[:, 0, 1], -Ah[:, 1, 2], -Ah[:, 0, 2],
                Amu[:, 0], Amu[:, 1], Amu[:, 2],
                -0.5 * muAmu + logop[hit]]).astype(np.float32)
            chi, clo = _bf16_split(c10)
            coef_m[0:10, o:o + nh] = chi
            coef_m[10:20, o:o + nh] = chi
            coef_m[20:30, o:o + nh] = clo
            # box penalties per dim (0 if inside, PEN outside)
            in_x = (np.abs(x0 + np.arange(BX)[None, :] - mx[hit, None])
                    <= radii[hit, None])
            in_y = (np.abs(y0 + np.arange(BY)[None, :] - my[hit, None])
                    <= radii[hit, None])
            in_z = (np.abs(z0 + np.arange(BZ)[None, :] - mz[hit, None])
                    <= radii[hit, None])
            coef_m[30:30 + BX, o:o + nh] = np.where(in_x, 0.0, PEN).T
            coef_m[30 + BX:30 + BX + BY, o:o + nh] = np.where(in_y, 0.0, PEN).T
            coef_m[30 + BX + BY:, o:o + nh] = np.where(in_z, 0.0, PEN).T
            sem_m[o:o + nh] = sem[hit].astype(np.float16)
        semp = np.zeros((128, NU * C), np.float16)
        for u, (g, off, Mt) in enumerate(units):
            semp[0:Mt, u * C:(u + 1) * C] = sem_m[off:off + Mt]
        perm_blocks.append(my_blocks)
        in_maps.append({"RHS": rhs, "COEF": coef_m.astype(ml_dtypes.bfloat16),
                        "SEMP": semp})
    return in_maps, L_slots, perm_blocks


def kernel(**inputs):
    in_maps, L_slots, perm_blocks = _host_prep(**inputs)
    nc = _get_nc(L_slots)
    run = _get_runner(nc)
    results, _, _ = run(in_maps)
    out = np.empty((N, C), np.float32)
    lx = np.arange(GPTS) // (BY * BZ)
    ly = (np.arange(GPTS) // BZ) % BY
    lz = np.arange(GPTS) % BZ
    for ci in range(NCORES):
        o = results[ci]["OUT"].astype(np.float32)   # [OROWS, NGRP*OCOLS]
        for g in range(NSLOT):
            bid = perm_blocks[ci][g]
            x0 = (bid // (NBY * NBZ)) * BX
            y0 = ((bid // NBZ) % NBY) * BY
            z0 = (bid % NBZ) * BZ
            gg = g % OGRP
            gi, s = gg // PERBANK, gg % PERBANK
            col0 = (g // OGRP) * OCOLS + s * GPTS
            blk = o[gi * 32:gi * 32 + C, col0:col0 + GPTS]   # [C, GPTS]
            gidx = ((x0 + lx) * W + (y0 + ly)) * D + (z0 + lz)
            out[gidx] = blk.T
    return out


# revision 13
# speedup vs baseline: 3.1257x; 1.0235x over previous
# Trainium2 Bass kernel for nn_LocalAggregator (Gaussian -> voxel-grid semantic
# compositing).
#
# Strategy: the voxel grid (60,60,36) is tiled into 1080 3-D blocks of
# (4,5,6) = 120 voxels.  Blocks are dealt to the 8 cores by sorted hit-count
# (rank 8k+c -> core c, slot k) so every core sees a near-identical load
# profile; the host un-permutes the output afterwards.  For each block the
# host builds the exact list of Gaussians whose integer box overlaps the
# block in all three dims.  In block-local coordinates the feature matrix
# (quadratic monomials + one-hot rows for the x/y/z box tests) is the SAME
# for every block, so a single small RHS is shared by all matmuls; all
# per-(block,Gaussian) data lives in the COEF matrix.  The E matmul runs in
# bf16 with a 3-way hi/lo product split (rows [hi,hi,lo] x features
# [fhi,flo,fhi] drop only the lo*lo term, ~1e-5 relative), Exp runs on the
# scalar engine over 12 units (3 PSUM banks x 4 blocks each) at a time, and
# the semantic reduction is an fp16 matmul accumulated into PSUM banks packed
# 12 blocks deep (3 partition stripes x 4 column slots) so one DVE copy +
# one Pool-engine DMA drains 12 blocks at once.  Dummy matmuls at t=0 ramp
# the PE clock to max p-state and a dummy activation preloads the Exp table.
import numpy as np
import ml_dtypes

H, W, D = 60, 60, 36
GRID = 0.08
SCALE_MULT = 3.0
P = 2048
C = 13
N = H * W * D                  # 129600
NCORES = 8
BX, BY, BZ = 4, 5, 6           # block shape
NBX, NBY, NBZ = H // BX, W // BY, D // BZ
NB = NBX * NBY * NBZ           # 1080 blocks total
GPTS = BX * BY * BZ            # 120 points per block
NSLOT = NB // NCORES           # 135 blocks (slots) per core
NPC = NSLOT * GPTS             # 16200 points per core
KF = 30 + BX + BY + BZ         # 45 feature rows (3x10 split products + onehots)
PEN = -2000.0                  # box-miss penalty (exp() == 0 in fp32)
PERBANK = 4                    # 120-col units per 512-f32 PSUM bank
NBANK = 3                      # banks per psE tile / act instruction
UPT = PERBANK * NBANK          # 12 units per psE tile
OGRP = 12                      # blocks per psO bank (3 stripes x 4 col slots)
OROWS = 77                     # 2*32+13 live partitions per output group
OCOLS = PERBANK * GPTS         # 480 cols per output group
NGRP = (NSLOT + OGRP - 1) // OGRP   # 12 output groups per core (last partial)

_NC_CACHE: dict = {}
_JIT_CACHE: dict = {}


def _build_nc(L_slots):
    import concourse.bacc as bacc
    import concourse.tile as tile
    from concourse import mybir

    L_slots = [int(x) for x in L_slots]
    offs = np.concatenate([[0], np.cumsum(L_slots)]).astype(int)
    Ltot = int(offs[-1])
    # units: (slot, coef col offset, Mt, first-in-slot, last-in-slot)
    units = []
    for g in range(NSLOT):
        L = L_slots[g]
        for s in range(0, L, 128):
            units.append((g, int(offs[g]) + s, min(128, L - s),
                          s == 0, s + 128 >= L))
    NU = len(units)
    tiles_u = [units[i:i + UPT] for i in range(0, NU, UPT)]

    nc = bacc.Bacc("TRN2", target_bir_lowering=False, debug=False,
                   num_devices=NCORES)
    f32 = mybir.dt.float32
    bf16 = mybir.dt.bfloat16
    f16 = mybir.dt.float16
    RHS = nc.dram_tensor("RHS", [KF, GPTS], bf16, kind="ExternalInput")
    COEF = nc.dram_tensor("COEF", [KF, Ltot], bf16, kind="ExternalInput")
    SEMP = nc.dram_tensor("SEMP", [128, NU * C], f16, kind="ExternalInput")
    OUT = nc.dram_tensor("OUT", [OROWS, NGRP * OCOLS], f16,
                         kind="ExternalOutput")

    # coef-chunk boundaries (slots): small first chunk so compute starts early
    CH_SLOTS = [0, 14, 55, NSLOT]
    # semp split: first piece covers the first ~2 chunks of units
    u_mid = sum(1 for u in units if u[0] < CH_SLOTS[1])

    with tile.TileContext(nc) as tc:
        with (
            tc.tile_pool(name="big", bufs=1) as big_pool,
            tc.tile_pool(name="w", bufs=4) as w_pool,
            tc.tile_pool(name="og", bufs=3) as og_pool,
            tc.tile_pool(name="psE", bufs=2, space="PSUM") as pse_pool,
            tc.tile_pool(name="psO", bufs=2, space="PSUM") as pso_pool,
        ):
            rhs_b = big_pool.tile([KF, GPTS], bf16)
            coef_b = big_pool.tile([KF, Ltot], bf16)
            semp_b = big_pool.tile([128, NU * C], f16)
            scr_b = big_pool.tile([1, 512], bf16)
            scr_o = big_pool.tile([1, 8], f16)

            # --- warmup: PE p-state ramp + activation table preload ---
            # (dummy ops read zeroed SBUF; results are never consumed)
            nc.vector.memset(scr_b[:], 0.0)
            nc.scalar.activation(scr_o[0:1, 0:1], scr_b[0:1, 0:1],
                                 mybir.ActivationFunctionType.Exp)
            psD = pse_pool.tile([128, NBANK, 512], f32, name="psD", tag="psE")
            for _ in range(4):
                nc.tensor.matmul(psD[0:1, 0:1, 0:512], scr_b[0:1, 0:1],
                                 scr_b[0:1, 0:512], start=True, stop=True,
                                 skip_group_check=True)

            # --- input loads: all on SP (HWDGE), first compute chunk first
            nc.sync.dma_start(rhs_b[:], RHS[:])
            a0, a1 = int(offs[CH_SLOTS[0]]), int(offs[CH_SLOTS[1]])
            nc.sync.dma_start(coef_b[:, a0:a1], COEF[:, a0:a1])
            nc.sync.dma_start(semp_b[:, 0:u_mid * C], SEMP[:, 0:u_mid * C])
            a1b, a2 = int(offs[CH_SLOTS[1]]), int(offs[CH_SLOTS[2]])
            nc.sync.dma_start(coef_b[:, a1b:a2], COEF[:, a1b:a2])
            nc.sync.dma_start(semp_b[:, u_mid * C:], SEMP[:, u_mid * C:])
            a2b, a3 = int(offs[CH_SLOTS[2]]), int(offs[CH_SLOTS[3]])
            nc.sync.dma_start(coef_b[:, a2b:a3], COEF[:, a2b:a3])

            pso_t = {}    # group -> psO tile
            uidx = 0
            for tu in tiles_u:
                psE = pse_pool.tile([128, NBANK, 512], f32, tag="psE")
                w_t = w_pool.tile([128, NBANK, OCOLS], f16)
                mtmax = max(u[2] for u in tu)
                for j, (g, off, Mt, first, last) in enumerate(tu):
                    b, s = j // PERBANK, j % PERBANK
                    nc.tensor.matmul(
                        psE[0:Mt, b:b + 1, s * GPTS:(s + 1) * GPTS],
                        coef_b[:, off:off + Mt], rhs_b[:],
                        start=True, stop=True, skip_group_check=True)
                nb_full, rem = divmod(len(tu), PERBANK)
                if nb_full:
                    nc.scalar.activation(
                        w_t[0:mtmax, 0:nb_full, :],
                        psE[0:mtmax, 0:nb_full, 0:OCOLS],
                        mybir.ActivationFunctionType.Exp)
                if rem:
                    nc.scalar.activation(
                        w_t[0:mtmax, nb_full:nb_full + 1, 0:rem * GPTS],
                        psE[0:mtmax, nb_full:nb_full + 1, 0:rem * GPTS],
                        mybir.ActivationFunctionType.Exp)
                for j, (g, off, Mt, first, last) in enumerate(tu):
                    grp, gg = g // OGRP, g % OGRP
                    gi, s = gg // PERBANK, gg % PERBANK
                    if first and gg == 0:
                        pso_t[grp] = pso_pool.tile([OROWS, OCOLS], f32,
                                                   name=f"psO{grp}",
                                                   tag="psO")
                    b, js = j // PERBANK, j % PERBANK
                    nc.tensor.matmul(
                        pso_t[grp][gi * 32:gi * 32 + C,
                                   s * GPTS:(s + 1) * GPTS],
                        semp_b[0:Mt, (uidx + j) * C:(uidx + j + 1) * C],
                        w_t[0:Mt, b:b + 1, js * GPTS:(js + 1) * GPTS],
                        start=first, stop=last, skip_group_check=True)
                    if last and (g == NSLOT - 1 or gg == OGRP - 1):
                        nblk = g % OGRP + 1
                        top = gg // PERBANK            # last stripe index
                        grows = top * 32 + C
                        gcols = OCOLS if top > 0 else nblk * GPTS
                        outg = og_pool.tile([OROWS, OCOLS], f16)
                        nc.vector.tensor_copy(outg[0:grows, 0:gcols],
                                              pso_t[grp][0:grows, 0:gcols])
                        dma_eng = (nc.gpsimd if grp < NGRP - 2 else nc.sync)
                        dma_eng.dma_start(
                            OUT[0:grows, grp * OCOLS:grp * OCOLS + gcols],
                            outg[0:grows, 0:gcols])
                uidx += len(tu)
    nc.compile()
    return nc


def _get_nc(L_slots):
    key = tuple(int(x) for x in L_slots)
    if key not in _NC_CACHE:
        _NC_CACHE[key] = _build_nc(L_slots)
    return _NC_CACHE[key]


def _get_runner(nc):
    """Cached shard_map-jitted executor for one Bass program (axon/PJRT path).

    Mirrors concourse.bass2jax.run_bass_via_pjrt but keeps the jitted callable
    so repeated runs don't rebuild/recompile."""
    if id(nc) in _JIT_CACHE:
        return _JIT_CACHE[id(nc)]
    import jax
    from concourse import bass2jax, mybir
    from jax.experimental.shard_map import shard_map
    from jax.sharding import Mesh, PartitionSpec

    bass2jax.install_neuronx_cc_hook()
    partition_name = (nc.partition_id_tensor.name
                      if nc.partition_id_tensor else None)
    in_names, out_names, out_avals, zero_outs = [], [], [], []
    for alloc in nc.m.functions[0].allocations:
        if not isinstance(alloc, mybir.MemoryLocationSet):
            continue
        name = alloc.memorylocations[0].name
        if alloc.kind == "ExternalInput":
            if name == partition_name:
                continue
            in_names.append(name)
        elif alloc.kind == "ExternalOutput":
            shape = tuple(alloc.tensor_shape)
            dtype = mybir.dt.np(alloc.dtype)
            out_names.append(name)
            out_avals.append(jax.core.ShapedArray(shape, dtype))
            zero_outs.append(np.zeros(shape, dtype))
    n_params = len(in_names)
    all_in_names = in_names + out_names
    if partition_name is not None:
        all_in_names = all_in_names + [partition_name]

    def _body(*args):
        operands = list(args)
        if partition_name is not None:
            operands.append(bass2jax.partition_id_tensor())
        outs = bass2jax._bass_exec_p.bind(
            *operands,
            out_avals=tuple(out_avals),
            in_names=tuple(all_in_names),
            out_names=tuple(out_names),
            lowering_input_output_aliases=(),
            sim_require_finite=True,
            sim_require_nnan=True,
            nc=nc,
        )
        return tuple(outs)

    devices = jax.devices()[:NCORES]
    mesh = Mesh(np.asarray(devices), ("core",))
    donate = tuple(range(n_params, n_params + len(out_names)))
    sharded = jax.jit(
        shard_map(_body, mesh=mesh,
                  in_specs=(PartitionSpec("core"),) * (n_params + len(out_names)),
                  out_specs=(PartitionSpec("core"),) * len(out_names),
                  check_rep=False),
        donate_argnums=donate, keep_unused=True)

    def run(in_maps, rounds=1):
        concat_in = [np.concatenate([np.asarray(m[nm]) for m in in_maps], axis=0)
                     for nm in in_names]
        outs = None
        for _ in range(rounds):
            zo = [np.concatenate([z] * NCORES, axis=0) for z in zero_outs]
            outs = sharded(*concat_in, *zo)
        outs = [np.asarray(o) for o in outs]
        results = []
        for ci in range(NCORES):
            d = {}
            for oi, nm in enumerate(out_names):
                per = outs[oi].shape[0] // NCORES
                d[nm] = outs[oi][ci * per:(ci + 1) * per]
            results.append(d)
        return results, sharded, (concat_in, zero_outs, in_names, out_names)

    sharded_nd = jax.jit(
        shard_map(_body, mesh=mesh,
                  in_specs=(PartitionSpec("core"),) * (n_params + len(out_names)),
                  out_specs=(PartitionSpec("core"),) * len(out_names),
                  check_rep=False),
        keep_unused=True)

    def timeit(in_maps, iters=30):
        import time as _time
        from jax.sharding import NamedSharding
        sh = NamedSharding(mesh, PartitionSpec("core"))
        concat_in = [np.concatenate([np.asarray(m[nm]) for m in in_maps], axis=0)
                     for nm in in_names]
        zo = [np.concatenate([z] * NCORES, axis=0) for z in zero_outs]
        args = [jax.device_put(a, sh) for a in concat_in + zo]
        outs = sharded_nd(*args)
        jax.block_until_ready(outs)
        t0 = _time.time()
        for _ in range(iters):
            outs = sharded_nd(*args)
        jax.block_until_ready(outs)
        return (_time.time() - t0) / iters

    run.timeit = timeit
    _JIT_CACHE[id(nc)] = run
    return run


def _bf16_split(x):
    hi = x.astype(ml_dtypes.bfloat16).astype(np.float32)
    return hi, (x - hi).astype(np.float32)


def _host_prep(pts, means3D, opacities, semantics, scales, cov3D, origin_use):
    pts = np.asarray(pts, np.float32).reshape(N, 3)
    mu32 = np.asarray(means3D, np.float32).reshape(P, 3)
    op = np.asarray(opacities, np.float64).reshape(P)
    sem = np.asarray(semantics, np.float32).reshape(P, C)
    sc32 = np.asarray(scales, np.float32).reshape(P, 3)
    cov = np.asarray(cov3D, np.float64).reshape(P, 3, 3)
    org32 = np.asarray(origin_use, np.float32).reshape(3)

    # --- integer binning, replicated in fp32 exactly like the reference ---
    radii = np.ceil(sc32.max(-1) * np.float32(SCALE_MULT) / np.float32(GRID)
                    ).astype(np.int32).astype(np.int64)
    m_int = ((mu32 - org32) / np.float32(GRID)).astype(np.int32).astype(np.int64)
    p_int = ((pts - org32) / np.float32(GRID)).astype(np.int32).astype(np.int64)

    # structured-input check: points must be the (i, j, k) voxel-center grid
    idx = np.arange(N)
    kk = idx % D
    col = idx // D
    jj = col % W
    ii = col // W
    grid_int = np.stack([ii, jj, kk], axis=-1)
    if not np.array_equal(p_int, grid_int):
        raise RuntimeError("kernel: unstructured pts not supported by fast path")

    # --- per-Gaussian inverse covariance (float64) ---
    a, b, c_, d, e, f = (cov[:, 0, 0], cov[:, 1, 1], cov[:, 2, 2],
                         cov[:, 0, 1], cov[:, 1, 2], cov[:, 0, 2])
    det = a * (b * c_ - e * e) - d * (d * c_ - e * f) + f * (d * e - b * f)
    ixx = (b * c_ - e * e) / det
    iyy = (a * c_ - f * f) / det
    izz = (a * b - d * d) / det
    ixy = (e * f - d * c_) / det
    iyz = (d * f - a * e) / det
    ixz = (d * e - b * f) / det
    A = np.empty((P, 3, 3))
    A[:, 0, 0], A[:, 1, 1], A[:, 2, 2] = ixx, iyy, izz
    A[:, 0, 1] = A[:, 1, 0] = ixy
    A[:, 1, 2] = A[:, 2, 1] = iyz
    A[:, 0, 2] = A[:, 2, 0] = ixz
    mu = mu32.astype(np.float64)
    logop = np.log(op)

    # --- blocks: bounds, hit lists, load-balanced assignment ---
    mx, my, mz = m_int[:, 0], m_int[:, 1], m_int[:, 2]
    blocks = []            # (x0, y0, z0)
    hits = []
    for bxi in range(NBX):
        for byi in range(NBY):
            for bzi in range(NBZ):
                x0, y0, z0 = bxi * BX, byi * BY, bzi * BZ
                hit = np.where(
                    (mx + radii >= x0) & (mx - radii <= x0 + BX - 1) &
                    (my + radii >= y0) & (my - radii <= y0 + BY - 1) &
                    (mz + radii >= z0) & (mz - radii <= z0 + BZ - 1))[0]
                blocks.append((x0, y0, z0))
                hits.append(hit)
    Ls = np.array([len(h) for h in hits])
    order = np.argsort(-Ls, kind="stable")     # rank r -> block id
    # core c, slot k gets block order[8k + c]; slot size = L of rank 8k
    L_slots = [max(1, int(Ls[order[8 * k]])) for k in range(NSLOT)]
    offs = np.concatenate([[0], np.cumsum(L_slots)]).astype(int)
    Ltot = int(offs[-1])
    units = []
    for g in range(NSLOT):
        L = L_slots[g]
        for s in range(0, L, 128):
            units.append((g, int(offs[g]) + s, min(128, L - s)))
    NU = len(units)

    # --- shared RHS: block-local features (hi/lo pairs) + one-hot rows ---
    lx = np.arange(GPTS) // (BY * BZ)
    ly = (np.arange(GPTS) // BZ) % BY
    lz = np.arange(GPTS) % BZ
    xi = ((lx - (BX - 1) / 2.0) * GRID).astype(np.float32)
    yi = ((ly - (BY - 1) / 2.0) * GRID).astype(np.float32)
    zi = ((lz - (BZ - 1) / 2.0) * GRID).astype(np.float32)
    feat10 = np.stack([xi * xi, yi * yi, zi * zi, xi * yi, yi * zi, xi * zi,
                       xi, yi, zi, np.ones(GPTS, np.float32)])
    fhi, flo = _bf16_split(feat10)
    rhs = np.zeros((KF, GPTS), np.float32)
    rhs[0:10] = fhi
    rhs[10:20] = flo
    rhs[20:30] = fhi
    rhs[30 + lx, np.arange(GPTS)] = 1.0
    rhs[30 + BX + ly, np.arange(GPTS)] = 1.0
    rhs[30 + BX + BY + lz, np.arange(GPTS)] = 1.0
    rhs = rhs.astype(ml_dtypes.bfloat16)

    # --- per-core COEF / SEMP ---
    in_maps = []
    perm_blocks = []       # per core: slot -> block id
    for ci in range(NCORES):
        coef_m = np.zeros((KF, Ltot), np.float32)
        sem_m = np.zeros((Ltot, C), np.float16)
        my_blocks = []
        for g in range(NSLOT):
            bid = int(order[8 * g + ci])
            my_blocks.append(bid)
            hit = hits[bid]
            nh = len(hit)
            if nh == 0:
                continue
            o = offs[g]
            x0, y0, z0 = blocks[bid]
            cx = (x0 + (BX - 1) / 2.0 + 0.5) * GRID
            cy = (y0 + (BY - 1) / 2.0 + 0.5) * GRID
            cz = (z0 + (BZ - 1) / 2.0 + 0.5) * GRID
            dmu = mu[hit] - np.array([cx, cy, cz])      # [nh, 3]
            Ah = A[hit]                                  # [nh, 3, 3]
            Amu = np.einsum('pij,pj->pi', Ah, dmu)
            muAmu = np.einsum('pi,pi->p', dmu, Amu)
            c10 = np.stack([
                -0.5 * Ah[:, 0, 0], -0.5 * Ah[:, 1, 1], -0.5 * Ah[:, 2, 2],
                -Ah[:, 0, 1], -Ah[:, 1, 2], -Ah[:, 0, 2],
                Amu[:, 0], Amu[:, 1], Amu[:, 2],
                -0.5 * muAmu + logop[hit]]).astype(np.float32)
            chi, clo = _bf16_split(c10)
            coef_m[0:10, o:o + nh] = chi
            coef_m[10:20, o:o + nh] = chi
            coef_m[20:30, o:o + nh] = clo
            # box penalties per dim (0 if inside, PEN outside)
            in_x = (np.abs(x0 + np.arange(BX)[None, :] - mx[hit, None])
                    <= radii[hit, None])
            in_y = (np.abs(y0 + np.arange(BY)[None, :] - my[hit, None])
                    <= radii[hit, None])
            in_z = (np.abs(z0 + np.arange(BZ)[None, :] - mz[hit, None])
                    <= radii[hit, None])
            coef_m[30:30 + BX, o:o + nh] = np.where(in_x, 0.0, PEN).T
            coef_m[30 + BX:30 + BX + BY, o:o + nh] = np.where(in_y, 0.0, PEN).T
            coef_m[30 + BX + BY:, o:o + nh] = np.where(in_z, 0.0, PEN).T
            sem_m[o:o + nh] = sem[hit].astype(np.float16)
        semp = np.zeros((128, NU * C), np.float16)
        for u, (g, off, Mt) in enumerate(units):
            semp[0:Mt, u * C:(u + 1) * C] = sem_m[off:off + Mt]
        perm_blocks.append(my_blocks)
        in_maps.append({"RHS": rhs, "COEF": coef_m.astype(ml_dtypes.bfloat16),
                        "SEMP": semp})
    return in_maps, L_slots, perm_blocks


def kernel(**inputs):
    in_maps, L_slots, perm_blocks = _host_prep(**inputs)
    nc = _get_nc(L_slots)
    run = _get_runner(nc)
    results, _, _ = run(in_maps)
    out = np.empty((N, C), np.float32)
    lx = np.arange(GPTS) // (BY * BZ)
    ly = (np.arange(GPTS) // BZ) % BY
    lz = np.arange(GPTS) % BZ
    for ci in range(NCORES):
        o = results[ci]["OUT"].astype(np.float32)   # [OROWS, NGRP*OCOLS]
        for g in range(NSLOT):
            bid = perm_blocks[ci][g]
            x0 = (bid // (NBY * NBZ)) * BX
            y0 = ((bid // NBZ) % NBY) * BY
            z0 = (bid % NBZ) * BZ
            gg = g % OGRP
            gi, s = gg // PERBANK, gg % PERBANK
            col0 = (g // OGRP) * OCOLS + s * GPTS
            blk = o[gi * 32:gi * 32 + C, col0:col0 + GPTS]   # [C, GPTS]
            gidx = ((x0 + lx) * W + (y0 + ly)) * D + (z0 + lz)
            out[gidx] = blk.T
    return out


# revision 31
# speedup vs baseline: 3.5314x; 1.1298x over previous
# Trainium2 Bass kernel for nn_LocalAggregator (Gaussian -> voxel-grid semantic
# compositing).
#
# Strategy: the voxel grid (60,60,36) is tiled into 1080 3-D blocks of
# (4,5,6) = 120 voxels.  Blocks are dealt to the 8 cores by sorted hit-count
# (rank 8k+c -> core c, slot k) so every core sees a near-identical load
# profile; the host un-permutes the output afterwards.  For each block the
# host builds the exact list of Gaussians whose integer box overlaps the
# block in all three dims.  In block-local coordinates the feature matrix
# (quadratic monomials + one-hot rows for the x/y/z box tests) is the SAME
# for every block, so a single small RHS is shared by all matmuls; all
# per-(block,Gaussian) data lives in the COEF matrix.  The E matmul runs in
# bf16 with a 3-way hi/lo product split (rows [hi,hi,lo] x features
# [fhi,flo,fhi] drop only the lo*lo term, ~1e-5 relative), Exp runs on the
# scalar engine over 12 units (3 PSUM banks x 4 blocks each) at a time, and
# the semantic reduction is an fp16 matmul accumulated into PSUM banks packed
# 12 blocks deep (3 partition stripes x 4 column slots) so one DVE copy +
# one Pool-engine DMA drains 12 blocks at once.  Dummy matmuls at t=0 ramp
# the PE clock to max p-state and a dummy activation preloads the Exp table.
import numpy as np
import ml_dtypes

H, W, D = 60, 60, 36
GRID = 0.08
SCALE_MULT = 3.0
P = 2048
C = 13
N = H * W * D                  # 129600
NCORES = 8
BX, BY, BZ = 4, 5, 6           # block shape
NBX, NBY, NBZ = H // BX, W // BY, D // BZ
NB = NBX * NBY * NBZ           # 1080 blocks total
GPTS = BX * BY * BZ            # 120 points per block
NSLOT = NB // NCORES           # 135 blocks (slots) per core
NPC = NSLOT * GPTS             # 16200 points per core
KF = 30 + BX + BY + BZ         # 45 feature rows (3x10 split products + onehots)
PEN = -2000.0                  # box-miss penalty (exp() == 0 in fp32)
PERBANK = 4                    # 120-col units per 512-f32 PSUM bank
NBANK = 2                      # banks per psE tile / act instruction
UPT = PERBANK * NBANK          # 12 units per psE tile
OGRP = 12                      # blocks per psO bank (3 stripes x 4 col slots)
OROWS = 77                     # 2*32+13 live partitions per output group
OCOLS = PERBANK * GPTS         # 480 cols per output group
NGRP = (NSLOT + OGRP - 1) // OGRP   # output groups per core (last partial)

_NC_CACHE: dict = {}
_JIT_CACHE: dict = {}


def _pack_units(L_slots):
    """units -> column-slots with partition stacking and load interleaving.

    Returns (cslots, grp_of, gg_of, gsz_of, Ltot) where cslots is a list of
    lists of (slot, coef_off, Mt, part_off, first_in_slot, last_in_slot, s0).
    COEF column offsets are assigned sequentially in processing order.
    Units with Mt<=64 stack 2-3 per column-slot at partition offsets
    {0, 32, 64} (matmul out/lhsT base partition constraint; offset 32 needs
    Mt<=32, 64 needs Mt<=64).  Packed column-slots are interleaved evenly
    between the single-unit ones so the PE load per activation tile stays
    flat.  psO groups are formed over the slot completion order.
    """
    singles, smalls = [], []
    for g, L in enumerate(L_slots):
        L = int(L)
        nparts = (L + 127) // 128
        part = []
        for s in range(0, L, 128):
            Mt = min(128, L - s)
            u = (g, Mt, s == 0, s + 128 >= L, s)
            if nparts > 1 or Mt > 64:
                part.append(u)
            else:
                smalls.append(u)
        if part:
            singles.append(part)      # parts of one slot stay adjacent
    multis, cur, curtop = [], None, 0
    for u in smalls:
        Mt = u[1]
        po = 64 * ((curtop + 63) // 64) if cur is not None else 0
        if False and (cur is not None and po + Mt <= 128 and
                (po == 0 or (po == 64 and Mt <= 64))):
            cur.append((u, po))
            curtop = po + Mt
        else:
            cur = [(u, 0)]
            curtop = Mt
            multis.append(cur)
    # interleave multis evenly among singles (keeping slot parts adjacent)
    seq = []          # list of cslots: list of ((g,Mt,first,last,s0), po)
    ns, nm = len(singles), len(multis)
    si = mi = 0
    acc = 0.0
    step = nm / max(1, ns + nm)
    while si < ns or mi < nm:
        acc += step
        if mi < nm and (acc >= 1.0 or si >= ns):
            seq.append(multis[mi]); mi += 1; acc -= 1.0
        elif si < ns:
            for u in singles[si]:
                seq.append([(u, 0)])
            si += 1
    # assign coef offsets in processing order
    cslots = []
    cum = 0
    for cs in seq:
        out = []
        for (g, Mt, first, last, s0), po in cs:
            out.append((g, cum, Mt, po, first, last, s0))
            cum += Mt
        cslots.append(out)
    Ltot = cum
    # slot completion order -> psO groups
    comp = {}
    for ci, cs in enumerate(cslots):
        for (g, off, Mt, po, first, last, s0) in cs:
            comp[g] = max(comp.get(g, -1), ci)
    order = sorted(range(len(L_slots)), key=lambda g: (comp[g], g))
    grp_of = [0] * len(L_slots)
    gg_of = [0] * len(L_slots)
    gsz_of = [0] * len(L_slots)
    nslot = len(L_slots)
    nfull = nslot // OGRP
    for pos, g in enumerate(order):
        q, rr = divmod(pos, OGRP)
        grp_of[g] = q
        gg_of[g] = rr
        gsz_of[g] = OGRP if q < nfull else nslot % OGRP
    return cslots, grp_of, gg_of, gsz_of, Ltot


def _build_nc(L_slots):
    import concourse.bacc as bacc
    import concourse.tile as tile
    from concourse import mybir

    L_slots = [int(x) for x in L_slots]
    cslots, grp_of, gg_of, gsz_of, Ltot = _pack_units(L_slots)
    units = [u for cs in cslots for u in cs]
    NU = len(units)
    NCS = len(cslots)
    tiles_u = [cslots[i:i + UPT] for i in range(0, NCS, UPT)]

    nc = bacc.Bacc("TRN2", target_bir_lowering=False, debug=False,
                   num_devices=NCORES)
    f32 = mybir.dt.float32
    bf16 = mybir.dt.bfloat16
    f16 = mybir.dt.float16
    COEF = nc.dram_tensor("COEF", [KF, GPTS + Ltot], bf16,
                          kind="ExternalInput")
    SEMP = nc.dram_tensor("SEMP", [128, NU * C], f16, kind="ExternalInput")
    OUT = nc.dram_tensor("OUT", [OROWS, NGRP * OCOLS], f16,
                         kind="ExternalOutput")

    usem = {}
    for ui_, u_ in enumerate(units):
        usem[(u_[0], u_[1])] = ui_

    # coef-chunk boundaries at flat unit indices (processing order)
    CH_U = [0, 13, 52, 100, NU]
    u_s0 = 52

    with tile.TileContext(nc) as tc:
        with (
            tc.tile_pool(name="big", bufs=1) as big_pool,
            tc.tile_pool(name="w", bufs=4) as w_pool,
            tc.tile_pool(name="og", bufs=3) as og_pool,
            tc.tile_pool(name="psE", bufs=3, space="PSUM") as pse_pool,
            tc.tile_pool(name="psO", bufs=2, space="PSUM") as pso_pool,
        ):
            coefx_b = big_pool.tile([KF, GPTS + Ltot], bf16)
            rhs_b = coefx_b[:, 0:GPTS]
            coef_b = coefx_b[:, GPTS:]
            semp_b = big_pool.tile([128, NU * C], f16)
            scr_b = big_pool.tile([1, 512], bf16)
            scr_o = big_pool.tile([1, 8], f16)

            # --- warmup: PE p-state ramp + activation table preload.
            # A cheap same-engine SEQ write seeds scr_b so the dummy
            # matmuls (which start the PE clock ramp) run immediately.
            nc.tensor.write(scr_b[0:1, 0:1],
                            np.zeros(1, ml_dtypes.bfloat16).tobytes())
            nc.scalar.activation(scr_o[0:1, 0:1], scr_b[0:1, 0:1],
                                 mybir.ActivationFunctionType.Exp)
            psD = pse_pool.tile([128, NBANK, 512], f32, name="psD", tag="psE")
            for _ in range(3):
                nc.tensor.matmul(psD[0:1, 0:1, 0:512], scr_b[0:1, 0:1],
                                 scr_b[0:1, 0:512], start=True, stop=True,
                                 skip_group_check=True)

            # --- input loads: all on SP (HWDGE); RHS rides with chunk 0
            cb = [units[u][1] if u < NU else Ltot for u in CH_U]
            nc.sync.dma_start(coefx_b[:, 0:GPTS + cb[1]],
                              COEF[:, 0:GPTS + cb[1]])
            nc.sync.dma_start(coef_b[:, cb[1]:cb[2]],
                              COEF[:, GPTS + cb[1]:GPTS + cb[2]])
            nc.sync.dma_start(semp_b[:, 0:u_s0 * C], SEMP[:, 0:u_s0 * C])
            nc.sync.dma_start(coef_b[:, cb[2]:cb[3]],
                              COEF[:, GPTS + cb[2]:GPTS + cb[3]])
            nc.sync.dma_start(semp_b[:, u_s0 * C:], SEMP[:, u_s0 * C:])
            nc.sync.dma_start(coef_b[:, cb[3]:cb[4]],
                              COEF[:, GPTS + cb[3]:GPTS + cb[4]])

            pso_t = {}    # group -> psO tile
            w_ts, psEs = {}, {}

            def emit_E(t):
                tu = tiles_u[t]
                psE = pse_pool.tile([128, NBANK, 512], f32, name=f"psE{t}",
                                    tag="psE")
                psEs[t] = psE
                for j, cs in enumerate(tu):
                    b, s = j // PERBANK, j % PERBANK
                    for (g, off, Mt, po, first, last, s0) in cs:
                        nc.tensor.matmul(
                            psE[po:po + Mt, b:b + 1,
                                s * GPTS:(s + 1) * GPTS],
                            coef_b[:, off:off + Mt], rhs_b[:],
                            start=True, stop=True, skip_group_check=True)

            def emit_act(t):
                tu = tiles_u[t]
                psE = psEs[t]
                w_t = w_pool.tile([128, NBANK, OCOLS], f16, name=f"w{t}",
                                  tag="w")
                w_ts[t] = w_t
                mtmax = max(u[3] + u[2] for cs in tu for u in cs)
                nb_full, rem = divmod(len(tu), PERBANK)
                if nb_full:
                    nc.scalar.activation(
                        w_t[0:mtmax, 0:nb_full, :],
                        psE[0:mtmax, 0:nb_full, 0:OCOLS],
                        mybir.ActivationFunctionType.Exp)
                if rem:
                    nc.scalar.activation(
                        w_t[0:mtmax, nb_full:nb_full + 1, 0:rem * GPTS],
                        psE[0:mtmax, nb_full:nb_full + 1, 0:rem * GPTS],
                        mybir.ActivationFunctionType.Exp)

            def emit_out(t):
                tu = tiles_u[t]
                w_t = w_ts[t]
                for j, cs in enumerate(tu):
                  b, js = j // PERBANK, j % PERBANK
                  for (g, off, Mt, po, first, last, s0) in cs:
                    grp, gg, gsz = grp_of[g], gg_of[g], gsz_of[g]
                    gi, s = gg // PERBANK, gg % PERBANK
                    ui = usem[(g, off)]
                    if first and gg == 0:
                        pso_t[grp] = pso_pool.tile([OROWS, OCOLS], f32,
                                                   name=f"psO{grp}",
                                                   tag="psO")
                    nc.tensor.matmul(
                        pso_t[grp][gi * 32:gi * 32 + C,
                                   s * GPTS:(s + 1) * GPTS],
                        semp_b[po:po + Mt, ui * C:(ui + 1) * C],
                        w_t[po:po + Mt, b:b + 1,
                            js * GPTS:(js + 1) * GPTS],
                        start=first, stop=last, skip_group_check=True)
                    if last and gg == gsz - 1:
                        top = gg // PERBANK            # last stripe index
                        grows = top * 32 + C
                        gcols = OCOLS if top > 0 else gsz * GPTS
                        outg = og_pool.tile([OROWS, OCOLS], f16,
                                            name=f"og{grp}", tag="og")
                        nc.vector.tensor_copy(outg[0:grows, 0:gcols],
                                              pso_t[grp][0:grows, 0:gcols])
                        dma_eng = nc.gpsimd if grp < NGRP - 2 else nc.sync
                        dma_eng.dma_start(
                            OUT[0:grows, grp * OCOLS:grp * OCOLS + gcols],
                            outg[0:grows, 0:gcols])

            emit_E(0)
            for t in range(len(tiles_u)):
                emit_act(t)
                if t + 1 < len(tiles_u):
                    emit_E(t + 1)
                emit_out(t)
    nc.compile()
    return nc


def _get_nc(L_slots):
    key = tuple(int(x) for x in L_slots)
    if key not in _NC_CACHE:
        _NC_CACHE[key] = _build_nc(L_slots)
    return _NC_CACHE[key]


def _get_runner(nc):
    """Cached shard_map-jitted executor for one Bass program (axon/PJRT path).

    Mirrors concourse.bass2jax.run_bass_via_pjrt but keeps the jitted callable
    so repeated runs don't rebuild/recompile."""
    if id(nc) in _JIT_CACHE:
        return _JIT_CACHE[id(nc)]
    import jax
    from concourse import bass2jax, mybir
    from jax.experimental.shard_map import shard_map
    from jax.sharding import Mesh, PartitionSpec

    bass2jax.install_neuronx_cc_hook()
    partition_name = (nc.partition_id_tensor.name
                      if nc.partition_id_tensor else None)
    in_names, out_names, out_avals, zero_outs = [], [], [], []
    for alloc in nc.m.functions[0].allocations:
        if not isinstance(alloc, mybir.MemoryLocationSet):
            continue
        name = alloc.memorylocations[0].name
        if alloc.kind == "ExternalInput":
            if name == partition_name:
                continue
            in_names.append(name)
        elif alloc.kind == "ExternalOutput":
            shape = tuple(alloc.tensor_shape)
            dtype = mybir.dt.np(alloc.dtype)
            out_names.append(name)
            out_avals.append(jax.core.ShapedArray(shape, dtype))
            zero_outs.append(np.zeros(shape, dtype))
    n_params = len(in_names)
    all_in_names = in_names + out_names
    if partition_name is not None:
        all_in_names = all_in_names + [partition_name]

    def _body(*args):
        operands = list(args)
        if partition_name is not None:
            operands.append(bass2jax.partition_id_tensor())
        outs = bass2jax._bass_exec_p.bind(
            *operands,
            out_avals=tuple(out_avals),
            in_names=tuple(all_in_names),
            out_names=tuple(out_names),
            lowering_input_output_aliases=(),
            sim_require_finite=True,
            sim_require_nnan=True,
            nc=nc,
        )
        return tuple(outs)

    devices = jax.devices()[:NCORES]
    mesh = Mesh(np.asarray(devices), ("core",))
    donate = tuple(range(n_params, n_params + len(out_names)))
    sharded = jax.jit(
        shard_map(_body, mesh=mesh,
                  in_specs=(PartitionSpec("core"),) * (n_params + len(out_names)),
                  out_specs=(PartitionSpec("core"),) * len(out_names),
                  check_rep=False),
        donate_argnums=donate, keep_unused=True)

    def run(in_maps, rounds=1):
        concat_in = [np.concatenate([np.asarray(m[nm]) for m in in_maps], axis=0)
                     for nm in in_names]
        outs = None
        for _ in range(rounds):
            zo = [np.concatenate([z] * NCORES, axis=0) for z in zero_outs]
            outs = sharded(*concat_in, *zo)
        outs = [np.asarray(o) for o in outs]
        results = []
        for ci in range(NCORES):
            d = {}
            for oi, nm in enumerate(out_names):
                per = outs[oi].shape[0] // NCORES
                d[nm] = outs[oi][ci * per:(ci + 1) * per]
            results.append(d)
        return results, sharded, (concat_in, zero_outs, in_names, out_names)

    sharded_nd = jax.jit(
        shard_map(_body, mesh=mesh,
                  in_specs=(PartitionSpec("core"),) * (n_params + len(out_names)),
                  out_specs=(PartitionSpec("core"),) * len(out_names),
                  check_rep=False),
        keep_unused=True)

    def timeit(in_maps, iters=30):
        import time as _time
        from jax.sharding import NamedSharding
        sh = NamedSharding(mesh, PartitionSpec("core"))
        concat_in = [np.concatenate([np.asarray(m[nm]) for m in in_maps], axis=0)
                     for nm in in_names]
        zo = [np.concatenate([z] * NCORES, axis=0) for z in zero_outs]
        args = [jax.device_put(a, sh) for a in concat_in + zo]
        outs = sharded_nd(*args)
        jax.block_until_ready(outs)
        t0 = _time.time()
        for _ in range(iters):
            outs = sharded_nd(*args)
        jax.block_until_ready(outs)
        return (_time.time() - t0) / iters

    run.timeit = timeit
    _JIT_CACHE[id(nc)] = run
    return run


def _bf16_split(x):
    hi = x.astype(ml_dtypes.bfloat16).astype(np.float32)
    return hi, (x - hi).astype(np.float32)


def _host_prep(pts, means3D, opacities, semantics, scales, cov3D, origin_use):
    pts = np.asarray(pts, np.float32).reshape(N, 3)
    mu32 = np.asarray(means3D, np.float32).reshape(P, 3)
    op = np.asarray(opacities, np.float64).reshape(P)
    sem = np.asarray(semantics, np.float32).reshape(P, C)
    sc32 = np.asarray(scales, np.float32).reshape(P, 3)
    cov = np.asarray(cov3D, np.float64).reshape(P, 3, 3)
    org32 = np.asarray(origin_use, np.float32).reshape(3)

    # --- integer binning, replicated in fp32 exactly like the reference ---
    radii = np.ceil(sc32.max(-1) * np.float32(SCALE_MULT) / np.float32(GRID)
                    ).astype(np.int32).astype(np.int64)
    m_int = ((mu32 - org32) / np.float32(GRID)).astype(np.int32).astype(np.int64)
    p_int = ((pts - org32) / np.float32(GRID)).astype(np.int32).astype(np.int64)

    # structured-input check: points must be the (i, j, k) voxel-center grid
    idx = np.arange(N)
    kk = idx % D
    col = idx // D
    jj = col % W
    ii = col // W
    grid_int = np.stack([ii, jj, kk], axis=-1)
    if not np.array_equal(p_int, grid_int):
        raise RuntimeError("kernel: unstructured pts not supported by fast path")

    # --- per-Gaussian inverse covariance (float64) ---
    a, b, c_, d, e, f = (cov[:, 0, 0], cov[:, 1, 1], cov[:, 2, 2],
                         cov[:, 0, 1], cov[:, 1, 2], cov[:, 0, 2])
    det = a * (b * c_ - e * e) - d * (d * c_ - e * f) + f * (d * e - b * f)
    ixx = (b * c_ - e * e) / det
    iyy = (a * c_ - f * f) / det
    izz = (a * b - d * d) / det
    ixy = (e * f - d * c_) / det
    iyz = (d * f - a * e) / det
    ixz = (d * e - b * f) / det
    A = np.empty((P, 3, 3))
    A[:, 0, 0], A[:, 1, 1], A[:, 2, 2] = ixx, iyy, izz
    A[:, 0, 1] = A[:, 1, 0] = ixy
    A[:, 1, 2] = A[:, 2, 1] = iyz
    A[:, 0, 2] = A[:, 2, 0] = ixz
    mu = mu32.astype(np.float64)
    logop = np.log(op)

    # --- blocks: bounds, hit lists, load-balanced assignment ---
    mx, my, mz = m_int[:, 0], m_int[:, 1], m_int[:, 2]
    blocks = []            # (x0, y0, z0)
    hits = []
    for bxi in range(NBX):
        for byi in range(NBY):
            for bzi in range(NBZ):
                x0, y0, z0 = bxi * BX, byi * BY, bzi * BZ
                hit = np.where(
                    (mx + radii >= x0) & (mx - radii <= x0 + BX - 1) &
                    (my + radii >= y0) & (my - radii <= y0 + BY - 1) &
                    (mz + radii >= z0) & (mz - radii <= z0 + BZ - 1))[0]
                blocks.append((x0, y0, z0))
                hits.append(hit)
    Ls = np.array([len(h) for h in hits])
    order = np.argsort(-Ls, kind="stable")     # rank r -> block id
    # core c, slot k gets block order[8k + c]; slot size = L of rank 8k
    L_slots = [max(1, int(Ls[order[8 * k]])) for k in range(NSLOT)]
    offs = np.concatenate([[0], np.cumsum(L_slots)]).astype(int)
    Ltot = int(offs[-1])
    cslots, grp_of, gg_of, gsz_of, Ltot = _pack_units(L_slots)
    units = [u for cs in cslots for u in cs]   # (g,off,Mt,po,first,last,s0)
    NU = len(units)

    # --- shared RHS: block-local features (hi/lo pairs) + one-hot rows ---
    lx = np.arange(GPTS) // (BY * BZ)
    ly = (np.arange(GPTS) // BZ) % BY
    lz = np.arange(GPTS) % BZ
    xi = ((lx - (BX - 1) / 2.0) * GRID).astype(np.float32)
    yi = ((ly - (BY - 1) / 2.0) * GRID).astype(np.float32)
    zi = ((lz - (BZ - 1) / 2.0) * GRID).astype(np.float32)
    feat10 = np.stack([xi * xi, yi * yi, zi * zi, xi * yi, yi * zi, xi * zi,
                       xi, yi, zi, np.ones(GPTS, np.float32)])
    fhi, flo = _bf16_split(feat10)
    rhs = np.zeros((KF, GPTS), np.float32)
    rhs[0:10] = fhi
    rhs[10:20] = flo
    rhs[20:30] = fhi
    rhs[30 + lx, np.arange(GPTS)] = 1.0
    rhs[30 + BX + ly, np.arange(GPTS)] = 1.0
    rhs[30 + BX + BY + lz, np.arange(GPTS)] = 1.0
    rhs = rhs.astype(ml_dtypes.bfloat16)

    # --- per-core COEF / SEMP ---
    in_maps = []
    perm_blocks = []       # per core: slot -> block id
    # per-slot units (in processing order they appear)
    units_of_slot = {}
    for u in units:
        units_of_slot.setdefault(u[0], []).append(u)
    for ci in range(NCORES):
        coef_m = np.zeros((KF, Ltot), np.float32)
        semp = np.zeros((128, NU * C), np.float16)
        my_blocks = []
        uindex = {}
        for ui, u in enumerate(units):
            uindex[(u[0], u[1])] = ui
        for g in range(NSLOT):
            bid = int(order[8 * g + ci])
            my_blocks.append(bid)
            hit = hits[bid]
            nh = len(hit)
            if nh == 0:
                continue
            x0, y0, z0 = blocks[bid]
            cx = (x0 + (BX - 1) / 2.0 + 0.5) * GRID
            cy = (y0 + (BY - 1) / 2.0 + 0.5) * GRID
            cz = (z0 + (BZ - 1) / 2.0 + 0.5) * GRID
            dmu = mu[hit] - np.array([cx, cy, cz])      # [nh, 3]
            Ah = A[hit]                                  # [nh, 3, 3]
            Amu = np.einsum('pij,pj->pi', Ah, dmu)
            muAmu = np.einsum('pi,pi->p', dmu, Amu)
            c10 = np.stack([
                -0.5 * Ah[:, 0, 0], -0.5 * Ah[:, 1, 1], -0.5 * Ah[:, 2, 2],
                -Ah[:, 0, 1], -Ah[:, 1, 2], -Ah[:, 0, 2],
                Amu[:, 0], Amu[:, 1], Amu[:, 2],
                -0.5 * muAmu + logop[hit]]).astype(np.float32)
            chi, clo = _bf16_split(c10)
            in_x = (np.abs(x0 + np.arange(BX)[None, :] - mx[hit, None])
                    <= radii[hit, None])
            in_y = (np.abs(y0 + np.arange(BY)[None, :] - my[hit, None])
                    <= radii[hit, None])
            in_z = (np.abs(z0 + np.arange(BZ)[None, :] - mz[hit, None])
                    <= radii[hit, None])
            px = np.where(in_x, 0.0, PEN).T
            py = np.where(in_y, 0.0, PEN).T
            pz = np.where(in_z, 0.0, PEN).T
            semh = sem[hit].astype(np.float16)
            for (gg_, off, Mt, po, first, last, s0) in units_of_slot[g]:
                a, b2 = s0, min(s0 + Mt, nh)
                if a >= nh:
                    continue
                w_ = b2 - a
                coef_m[0:10, off:off + w_] = chi[:, a:b2]
                coef_m[10:20, off:off + w_] = chi[:, a:b2]
                coef_m[20:30, off:off + w_] = clo[:, a:b2]
                coef_m[30:30 + BX, off:off + w_] = px[:, a:b2]
                coef_m[30 + BX:30 + BX + BY, off:off + w_] = py[:, a:b2]
                coef_m[30 + BX + BY:, off:off + w_] = pz[:, a:b2]
                ui = uindex[(gg_, off)]
                semp[po:po + w_, ui * C:(ui + 1) * C] = semh[a:b2]
        perm_blocks.append(my_blocks)
        coefx = np.concatenate([np.asarray(rhs, np.float32),
                                coef_m], axis=1).astype(ml_dtypes.bfloat16)
        in_maps.append({"COEF": coefx, "SEMP": semp})
    return in_maps, L_slots, (perm_blocks, grp_of, gg_of)


def kernel(**inputs):
    in_maps, L_slots, (perm_blocks, grp_of, gg_of) = _host_prep(**inputs)
    nc = _get_nc(L_slots)
    run = _get_runner(nc)
    results, _, _ = run(in_maps)
    out = np.empty((N, C), np.float32)
    lx = np.arange(GPTS) // (BY * BZ)
    ly = (np.arange(GPTS) // BZ) % BY
    lz = np.arange(GPTS) % BZ
    for ci in range(NCORES):
        o = results[ci]["OUT"].astype(np.float32)   # [OROWS, NGRP*OCOLS]
        for g in range(NSLOT):
            bid = perm_blocks[ci][g]
            x0 = (bid // (NBY * NBZ)) * BX
            y0 = ((bid // NBZ) % NBY) * BY
            z0 = (bid % NBZ) * BZ
            grp, gg = grp_of[g], gg_of[g]
            gi, s = gg // PERBANK, gg % PERBANK
            col0 = grp * OCOLS + s * GPTS
            blk = o[gi * 32:gi * 32 + C, col0:col0 + GPTS]   # [C, GPTS]
            gidx = ((x0 + lx) * W + (y0 + ly)) * D + (z0 + lz)
            out[gidx] = blk.T
    return out


# revision 32
# speedup vs baseline: 3.7283x; 1.0558x over previous
# Trainium2 Bass kernel for nn_LocalAggregator (Gaussian -> voxel-grid semantic
# compositing).
#
# Strategy: the voxel grid (60,60,36) is tiled into 1080 3-D blocks of
# (4,5,6) = 120 voxels.  Blocks are dealt to the 8 cores by sorted hit-count
# (rank 8k+c -> core c, slot k) so every core sees a near-identical load
# profile; the host un-permutes the output afterwards.  For each block the
# host builds the exact list of Gaussians whose integer box overlaps the
# block in all three dims.  In block-local coordinates the feature matrix
# (quadratic monomials + one-hot rows for the x/y/z box tests) is the SAME
# for every block, so a single small RHS is shared by all matmuls; all
# per-(block,Gaussian) data lives in the COEF matrix.  The E matmul runs in
# bf16 with a 3-way hi/lo product split (rows [hi,hi,lo] x features
# [fhi,flo,fhi] drop only the lo*lo term, ~1e-5 relative), Exp runs on the
# scalar engine over 12 units (3 PSUM banks x 4 blocks each) at a time, and
# the semantic reduction is an fp16 matmul accumulated into PSUM banks packed
# 12 blocks deep (3 partition stripes x 4 column slots) so one DVE copy +
# one Pool-engine DMA drains 12 blocks at once.  Dummy matmuls at t=0 ramp
# the PE clock to max p-state and a dummy activation preloads the Exp table.
import numpy as np
import ml_dtypes

H, W, D = 60, 60, 36
GRID = 0.08
SCALE_MULT = 3.0
P = 2048
C = 13
N = H * W * D                  # 129600
NCORES = 8
BX, BY, BZ = 4, 5, 6           # block shape
NBX, NBY, NBZ = H // BX, W // BY, D // BZ
NB = NBX * NBY * NBZ           # 1080 blocks total
GPTS = BX * BY * BZ            # 120 points per block
NSLOT = NB // NCORES           # 135 blocks (slots) per core
NPC = NSLOT * GPTS             # 16200 points per core
KF = 30 + BX + BY + BZ         # 45 feature rows (3x10 split products + onehots)
PEN = -2000.0                  # box-miss penalty (exp() == 0 in fp32)
PERBANK = 4                    # 120-col units per 512-f32 PSUM bank
NBANK = 2                      # banks per psE tile / act instruction
UPT = PERBANK * NBANK          # 12 units per psE tile
OGRP = 12                      # blocks per psO bank (3 stripes x 4 col slots)
OROWS = 77                     # 2*32+13 live partitions per output group
OCOLS = PERBANK * GPTS         # 480 cols per output group
NGRP = (NSLOT + OGRP - 1) // OGRP   # output groups per core (last partial)

_NC_CACHE: dict = {}
_JIT_CACHE: dict = {}


def _pack_units(L_slots):
    """units -> column-slots with partition stacking and load interleaving.

    Returns (cslots, grp_of, gg_of, gsz_of, Ltot) where cslots is a list of
    lists of (slot, coef_off, Mt, part_off, first_in_slot, last_in_slot, s0).
    COEF column offsets are assigned sequentially in processing order.
    Units with Mt<=64 stack 2-3 per column-slot at partition offsets
    {0, 32, 64} (matmul out/lhsT base partition constraint; offset 32 needs
    Mt<=32, 64 needs Mt<=64).  Packed column-slots are interleaved evenly
    between the single-unit ones so the PE load per activation tile stays
    flat.  psO groups are formed over the slot completion order.
    """
    singles, smalls = [], []
    for g, L in enumerate(L_slots):
        L = int(L)
        nparts = (L + 127) // 128
        part = []
        for s in range(0, L, 128):
            Mt = min(128, L - s)
            u = (g, Mt, s == 0, s + 128 >= L, s)
            if nparts > 1 or Mt > 64:
                part.append(u)
            else:
                smalls.append(u)
        if part:
            singles.append(part)      # parts of one slot stay adjacent
    # zero-padded pairs: both members padded to 64 rows; the out-matmul
    # then contracts the full 128 partitions at base 0 and the zero
    # semantics rows mask the partner block.
    multis = []
    for i in range(0, len(smalls) - 1, 2):
        multis.append([(smalls[i], 0, True), (smalls[i + 1], 64, True)])
    if len(smalls) % 2:
        multis.append([(smalls[-1], 0, False)])
    # interleave multis evenly among singles (keeping slot parts adjacent)
    seq = []          # list of cslots: list of ((g,Mt,first,last,s0), po)
    ns, nm = len(singles), len(multis)
    si = mi = 0
    acc = 0.0
    step = nm / max(1, ns + nm)
    while si < ns or mi < nm:
        acc += step
        if mi < nm and (acc >= 1.0 or si >= ns):
            seq.append(multis[mi]); mi += 1; acc -= 1.0
        elif si < ns:
            for u in singles[si]:
                seq.append([(u, 0, False)])
            si += 1
    # assign coef offsets in processing order; padded units reserve the
    # full 64 columns (zero coef -> E=0 -> w=1, zero semantics -> no-op)
    cslots = []
    cum = 0
    for cs in seq:
        out = []
        for (g, Mt, first, last, s0), po, padded in cs:
            w = 64 if padded else Mt
            out.append((g, cum, w, po, first, last, s0, padded))
            cum += w
        cslots.append(out)
    Ltot = cum
    # slot completion order -> psO groups
    comp = {}
    for ci, cs in enumerate(cslots):
        for (g, off, Mt, po, first, last, s0, pd) in cs:
            comp[g] = max(comp.get(g, -1), ci)
    order = sorted(range(len(L_slots)), key=lambda g: (comp[g], g))
    grp_of = [0] * len(L_slots)
    gg_of = [0] * len(L_slots)
    gsz_of = [0] * len(L_slots)
    nslot = len(L_slots)
    nfull = nslot // OGRP
    for pos, g in enumerate(order):
        q, rr = divmod(pos, OGRP)
        grp_of[g] = q
        gg_of[g] = rr
        gsz_of[g] = OGRP if q < nfull else nslot % OGRP
    return cslots, grp_of, gg_of, gsz_of, Ltot


def _build_nc(L_slots):
    import concourse.bacc as bacc
    import concourse.tile as tile
    from concourse import mybir

    L_slots = [int(x) for x in L_slots]
    cslots, grp_of, gg_of, gsz_of, Ltot = _pack_units(L_slots)
    units = [u for cs in cslots for u in cs]
    NU = len(units)
    NCS = len(cslots)
    tiles_u = [cslots[i:i + UPT] for i in range(0, NCS, UPT)]

    nc = bacc.Bacc("TRN2", target_bir_lowering=False, debug=False,
                   num_devices=NCORES)
    f32 = mybir.dt.float32
    bf16 = mybir.dt.bfloat16
    f16 = mybir.dt.float16
    COEF = nc.dram_tensor("COEF", [KF, GPTS + Ltot], bf16,
                          kind="ExternalInput")
    SEMP = nc.dram_tensor("SEMP", [128, NU * C], f16, kind="ExternalInput")
    OUT = nc.dram_tensor("OUT", [OROWS, NGRP * OCOLS], f16,
                         kind="ExternalOutput")

    usem = {}
    for ui_, u_ in enumerate(units):
        usem[(u_[0], u_[1])] = ui_

    # coef-chunk boundaries at flat unit indices (processing order)
    CH_U = [0, 13, 52, 100, NU]
    u_s0 = 52

    with tile.TileContext(nc) as tc:
        with (
            tc.tile_pool(name="big", bufs=1) as big_pool,
            tc.tile_pool(name="w", bufs=4) as w_pool,
            tc.tile_pool(name="og", bufs=3) as og_pool,
            tc.tile_pool(name="psE", bufs=3, space="PSUM") as pse_pool,
            tc.tile_pool(name="psO", bufs=2, space="PSUM") as pso_pool,
        ):
            coefx_b = big_pool.tile([KF, GPTS + Ltot], bf16)
            rhs_b = coefx_b[:, 0:GPTS]
            coef_b = coefx_b[:, GPTS:]
            semp_b = big_pool.tile([128, NU * C], f16)
            scr_b = big_pool.tile([1, 512], bf16)
            scr_o = big_pool.tile([1, 8], f16)

            # --- warmup: PE p-state ramp + activation table preload.
            # A cheap same-engine SEQ write seeds scr_b so the dummy
            # matmuls (which start the PE clock ramp) run immediately.
            nc.tensor.write(scr_b[0:1, 0:1],
                            np.zeros(1, ml_dtypes.bfloat16).tobytes())
            nc.scalar.activation(scr_o[0:1, 0:1], scr_b[0:1, 0:1],
                                 mybir.ActivationFunctionType.Exp)
            psD = pse_pool.tile([128, NBANK, 512], f32, name="psD", tag="psE")
            for _ in range(3):
                nc.tensor.matmul(psD[0:1, 0:1, 0:512], scr_b[0:1, 0:1],
                                 scr_b[0:1, 0:512], start=True, stop=True,
                                 skip_group_check=True)

            # --- input loads: all on SP (HWDGE); RHS rides with chunk 0
            cb = [units[u][1] if u < NU else Ltot for u in CH_U]
            nc.sync.dma_start(coefx_b[:, 0:GPTS + cb[1]],
                              COEF[:, 0:GPTS + cb[1]])
            nc.sync.dma_start(coef_b[:, cb[1]:cb[2]],
                              COEF[:, GPTS + cb[1]:GPTS + cb[2]])
            nc.sync.dma_start(semp_b[:, 0:u_s0 * C], SEMP[:, 0:u_s0 * C])
            nc.sync.dma_start(coef_b[:, cb[2]:cb[3]],
                              COEF[:, GPTS + cb[2]:GPTS + cb[3]])
            nc.sync.dma_start(semp_b[:, u_s0 * C:], SEMP[:, u_s0 * C:])
            nc.sync.dma_start(coef_b[:, cb[3]:cb[4]],
                              COEF[:, GPTS + cb[3]:GPTS + cb[4]])

            pso_t = {}    # group -> psO tile
            w_ts, psEs = {}, {}

            def emit_E(t):
                tu = tiles_u[t]
                psE = pse_pool.tile([128, NBANK, 512], f32, name=f"psE{t}",
                                    tag="psE")
                psEs[t] = psE
                for j, cs in enumerate(tu):
                    b, s = j // PERBANK, j % PERBANK
                    for (g, off, Mt, po, first, last, s0, pd) in cs:
                        nc.tensor.matmul(
                            psE[po:po + Mt, b:b + 1,
                                s * GPTS:(s + 1) * GPTS],
                            coef_b[:, off:off + Mt], rhs_b[:],
                            start=True, stop=True, skip_group_check=True)

            def emit_act(t):
                tu = tiles_u[t]
                psE = psEs[t]
                w_t = w_pool.tile([128, NBANK, OCOLS], f16, name=f"w{t}",
                                  tag="w")
                w_ts[t] = w_t
                mtmax = max(u[3] + u[2] for cs in tu for u in cs)
                nb_full, rem = divmod(len(tu), PERBANK)
                if nb_full:
                    nc.scalar.activation(
                        w_t[0:mtmax, 0:nb_full, :],
                        psE[0:mtmax, 0:nb_full, 0:OCOLS],
                        mybir.ActivationFunctionType.Exp)
                if rem:
                    nc.scalar.activation(
                        w_t[0:mtmax, nb_full:nb_full + 1, 0:rem * GPTS],
                        psE[0:mtmax, nb_full:nb_full + 1, 0:rem * GPTS],
                        mybir.ActivationFunctionType.Exp)

            def emit_out(t):
                tu = tiles_u[t]
                w_t = w_ts[t]
                for j, cs in enumerate(tu):
                  b, js = j // PERBANK, j % PERBANK
                  for (g, off, Mt, po, first, last, s0, pd) in cs:
                    grp, gg, gsz = grp_of[g], gg_of[g], gsz_of[g]
                    gi, s = gg // PERBANK, gg % PERBANK
                    ui = usem[(g, off)]
                    if first and gg == 0:
                        pso_t[grp] = pso_pool.tile([OROWS, OCOLS], f32,
                                                   name=f"psO{grp}",
                                                   tag="psO")
                    k0, k1 = (0, 128) if pd else (0, Mt)
                    nc.tensor.matmul(
                        pso_t[grp][gi * 32:gi * 32 + C,
                                   s * GPTS:(s + 1) * GPTS],
                        semp_b[k0:k1, ui * C:(ui + 1) * C],
                        w_t[k0:k1, b:b + 1,
                            js * GPTS:(js + 1) * GPTS],
                        start=first, stop=last, skip_group_check=True)
                    if last and gg == gsz - 1:
                        top = gg // PERBANK            # last stripe index
                        grows = top * 32 + C
                        gcols = OCOLS if top > 0 else gsz * GPTS
                        outg = og_pool.tile([OROWS, OCOLS], f16,
                                            name=f"og{grp}", tag="og")
                        nc.vector.tensor_copy(outg[0:grows, 0:gcols],
                                              pso_t[grp][0:grows, 0:gcols])
                        dma_eng = nc.gpsimd if grp < NGRP - 2 else nc.sync
                        dma_eng.dma_start(
                            OUT[0:grows, grp * OCOLS:grp * OCOLS + gcols],
                            outg[0:grows, 0:gcols])

            emit_E(0)
            for t in range(len(tiles_u)):
                emit_act(t)
                if t + 1 < len(tiles_u):
                    emit_E(t + 1)
                emit_out(t)
    nc.compile()
    return nc


def _get_nc(L_slots):
    key = tuple(int(x) for x in L_slots)
    if key not in _NC_CACHE:
        _NC_CACHE[key] = _build_nc(L_slots)
    return _NC_CACHE[key]


def _get_runner(nc):
    """Cached shard_map-jitted executor for one Bass program (axon/PJRT path).

    Mirrors concourse.bass2jax.run_bass_via_pjrt but keeps the jitted callable
    so repeated runs don't rebuild/recompile."""
    if id(nc) in _JIT_CACHE:
        return _JIT_CACHE[id(nc)]
    import jax
    from concourse import bass2jax, mybir
    from jax.experimental.shard_map import shard_map
    from jax.sharding import Mesh, PartitionSpec

    bass2jax.install_neuronx_cc_hook()
    partition_name = (nc.partition_id_tensor.name
                      if nc.partition_id_tensor else None)
    in_names, out_names, out_avals, zero_outs = [], [], [], []
    for alloc in nc.m.functions[0].allocations:
        if not isinstance(alloc, mybir.MemoryLocationSet):
            continue
        name = alloc.memorylocations[0].name
        if alloc.kind == "ExternalInput":
            if name == partition_name:
                continue
            in_names.append(name)
        elif alloc.kind == "ExternalOutput":
            shape = tuple(alloc.tensor_shape)
            dtype = mybir.dt.np(alloc.dtype)
            out_names.append(name)
            out_avals.append(jax.core.ShapedArray(shape, dtype))
            zero_outs.append(np.zeros(shape, dtype))
    n_params = len(in_names)
    all_in_names = in_names + out_names
    if partition_name is not None:
        all_in_names = all_in_names + [partition_name]

    def _body(*args):
        operands = list(args)
        if partition_name is not None:
            operands.append(bass2jax.partition_id_tensor())
        outs = bass2jax._bass_exec_p.bind(
            *operands,
            out_avals=tuple(out_avals),
            in_names=tuple(all_in_names),
            out_names=tuple(out_names),
            lowering_input_output_aliases=(),
            sim_require_finite=True,
            sim_require_nnan=True,
            nc=nc,
        )
        return tuple(outs)

    devices = jax.devices()[:NCORES]
    mesh = Mesh(np.asarray(devices), ("core",))
    donate = tuple(range(n_params, n_params + len(out_names)))
    sharded = jax.jit(
        shard_map(_body, mesh=mesh,
                  in_specs=(PartitionSpec("core"),) * (n_params + len(out_names)),
                  out_specs=(PartitionSpec("core"),) * len(out_names),
                  check_rep=False),
        donate_argnums=donate, keep_unused=True)

    def run(in_maps, rounds=1):
        concat_in = [np.concatenate([np.asarray(m[nm]) for m in in_maps], axis=0)
                     for nm in in_names]
        outs = None
        for _ in range(rounds):
            zo = [np.concatenate([z] * NCORES, axis=0) for z in zero_outs]
            outs = sharded(*concat_in, *zo)
        outs = [np.asarray(o) for o in outs]
        results = []
        for ci in range(NCORES):
            d = {}
            for oi, nm in enumerate(out_names):
                per = outs[oi].shape[0] // NCORES
                d[nm] = outs[oi][ci * per:(ci + 1) * per]
            results.append(d)
        return results, sharded, (concat_in, zero_outs, in_names, out_names)

    sharded_nd = jax.jit(
        shard_map(_body, mesh=mesh,
                  in_specs=(PartitionSpec("core"),) * (n_params + len(out_names)),
                  out_specs=(PartitionSpec("core"),) * len(out_names),
                  check_rep=False),
        keep_unused=True)

    def timeit(in_maps, iters=30):
        import time as _time
        from jax.sharding import NamedSharding
        sh = NamedSharding(mesh, PartitionSpec("core"))
        concat_in = [np.concatenate([np.asarray(m[nm]) for m in in_maps], axis=0)
                     for nm in in_names]
        zo = [np.concatenate([z] * NCORES, axis=0) for z in zero_outs]
        args = [jax.device_put(a, sh) for a in concat_in + zo]
        outs = sharded_nd(*args)
        jax.block_until_ready(outs)
        t0 = _time.time()
        for _ in range(iters):
            outs = sharded_nd(*args)
        jax.block_until_ready(outs)
        return (_time.time() - t0) / iters

    run.timeit = timeit
    _JIT_CACHE[id(nc)] = run
    return run


def _bf16_split(x):
    hi = x.astype(ml_dtypes.bfloat16).astype(np.float32)
    return hi, (x - hi).astype(np.float32)


def _host_prep(pts, means3D, opacities, semantics, scales, cov3D, origin_use):
    pts = np.asarray(pts, np.float32).reshape(N, 3)
    mu32 = np.asarray(means3D, np.float32).reshape(P, 3)
    op = np.asarray(opacities, np.float64).reshape(P)
    sem = np.asarray(semantics, np.float32).reshape(P, C)
    sc32 = np.asarray(scales, np.float32).reshape(P, 3)
    cov = np.asarray(cov3D, np.float64).reshape(P, 3, 3)
    org32 = np.asarray(origin_use, np.float32).reshape(3)

    # --- integer binning, replicated in fp32 exactly like the reference ---
    radii = np.ceil(sc32.max(-1) * np.float32(SCALE_MULT) / np.float32(GRID)
                    ).astype(np.int32).astype(np.int64)
    m_int = ((mu32 - org32) / np.float32(GRID)).astype(np.int32).astype(np.int64)
    p_int = ((pts - org32) / np.float32(GRID)).astype(np.int32).astype(np.int64)

    # structured-input check: points must be the (i, j, k) voxel-center grid
    idx = np.arange(N)
    kk = idx % D
    col = idx // D
    jj = col % W
    ii = col // W
    grid_int = np.stack([ii, jj, kk], axis=-1)
    if not np.array_equal(p_int, grid_int):
        raise RuntimeError("kernel: unstructured pts not supported by fast path")

    # --- per-Gaussian inverse covariance (float64) ---
    a, b, c_, d, e, f = (cov[:, 0, 0], cov[:, 1, 1], cov[:, 2, 2],
                         cov[:, 0, 1], cov[:, 1, 2], cov[:, 0, 2])
    det = a * (b * c_ - e * e) - d * (d * c_ - e * f) + f * (d * e - b * f)
    ixx = (b * c_ - e * e) / det
    iyy = (a * c_ - f * f) / det
    izz = (a * b - d * d) / det
    ixy = (e * f - d * c_) / det
    iyz = (d * f - a * e) / det
    ixz = (d * e - b * f) / det
    A = np.empty((P, 3, 3))
    A[:, 0, 0], A[:, 1, 1], A[:, 2, 2] = ixx, iyy, izz
    A[:, 0, 1] = A[:, 1, 0] = ixy
    A[:, 1, 2] = A[:, 2, 1] = iyz
    A[:, 0, 2] = A[:, 2, 0] = ixz
    mu = mu32.astype(np.float64)
    logop = np.log(op)

    # --- blocks: bounds, hit lists, load-balanced assignment ---
    mx, my, mz = m_int[:, 0], m_int[:, 1], m_int[:, 2]
    blocks = []            # (x0, y0, z0)
    hits = []
    for bxi in range(NBX):
        for byi in range(NBY):
            for bzi in range(NBZ):
                x0, y0, z0 = bxi * BX, byi * BY, bzi * BZ
                hit = np.where(
                    (mx + radii >= x0) & (mx - radii <= x0 + BX - 1) &
                    (my + radii >= y0) & (my - radii <= y0 + BY - 1) &
                    (mz + radii >= z0) & (mz - radii <= z0 + BZ - 1))[0]
                blocks.append((x0, y0, z0))
                hits.append(hit)
    Ls = np.array([len(h) for h in hits])
    order = np.argsort(-Ls, kind="stable")     # rank r -> block id
    # core c, slot k gets block order[8k + c]; slot size = L of rank 8k
    L_slots = [max(1, int(Ls[order[8 * k]])) for k in range(NSLOT)]
    offs = np.concatenate([[0], np.cumsum(L_slots)]).astype(int)
    Ltot = int(offs[-1])
    cslots, grp_of, gg_of, gsz_of, Ltot = _pack_units(L_slots)
    units = [u for cs in cslots for u in cs]
    NU = len(units)

    # --- shared RHS: block-local features (hi/lo pairs) + one-hot rows ---
    lx = np.arange(GPTS) // (BY * BZ)
    ly = (np.arange(GPTS) // BZ) % BY
    lz = np.arange(GPTS) % BZ
    xi = ((lx - (BX - 1) / 2.0) * GRID).astype(np.float32)
    yi = ((ly - (BY - 1) / 2.0) * GRID).astype(np.float32)
    zi = ((lz - (BZ - 1) / 2.0) * GRID).astype(np.float32)
    feat10 = np.stack([xi * xi, yi * yi, zi * zi, xi * yi, yi * zi, xi * zi,
                       xi, yi, zi, np.ones(GPTS, np.float32)])
    fhi, flo = _bf16_split(feat10)
    rhs = np.zeros((KF, GPTS), np.float32)
    rhs[0:10] = fhi
    rhs[10:20] = flo
    rhs[20:30] = fhi
    rhs[30 + lx, np.arange(GPTS)] = 1.0
    rhs[30 + BX + ly, np.arange(GPTS)] = 1.0
    rhs[30 + BX + BY + lz, np.arange(GPTS)] = 1.0
    rhs = rhs.astype(ml_dtypes.bfloat16)

    # --- per-core COEF / SEMP ---
    in_maps = []
    perm_blocks = []       # per core: slot -> block id
    # per-slot units (in processing order they appear)
    units_of_slot = {}
    for u in units:
        units_of_slot.setdefault(u[0], []).append(u)
    for ci in range(NCORES):
        coef_m = np.zeros((KF, Ltot), np.float32)
        semp = np.zeros((128, NU * C), np.float16)
        my_blocks = []
        uindex = {}
        for ui, u in enumerate(units):
            uindex[(u[0], u[1])] = ui
        for g in range(NSLOT):
            bid = int(order[8 * g + ci])
            my_blocks.append(bid)
            hit = hits[bid]
            nh = len(hit)
            if nh == 0:
                continue
            x0, y0, z0 = blocks[bid]
            cx = (x0 + (BX - 1) / 2.0 + 0.5) * GRID
            cy = (y0 + (BY - 1) / 2.0 + 0.5) * GRID
            cz = (z0 + (BZ - 1) / 2.0 + 0.5) * GRID
            dmu = mu[hit] - np.array([cx, cy, cz])      # [nh, 3]
            Ah = A[hit]                                  # [nh, 3, 3]
            Amu = np.einsum('pij,pj->pi', Ah, dmu)
            muAmu = np.einsum('pi,pi->p', dmu, Amu)
            c10 = np.stack([
                -0.5 * Ah[:, 0, 0], -0.5 * Ah[:, 1, 1], -0.5 * Ah[:, 2, 2],
                -Ah[:, 0, 1], -Ah[:, 1, 2], -Ah[:, 0, 2],
                Amu[:, 0], Amu[:, 1], Amu[:, 2],
                -0.5 * muAmu + logop[hit]]).astype(np.float32)
            chi, clo = _bf16_split(c10)
            in_x = (np.abs(x0 + np.arange(BX)[None, :] - mx[hit, None])
                    <= radii[hit, None])
            in_y = (np.abs(y0 + np.arange(BY)[None, :] - my[hit, None])
                    <= radii[hit, None])
            in_z = (np.abs(z0 + np.arange(BZ)[None, :] - mz[hit, None])
                    <= radii[hit, None])
            px = np.where(in_x, 0.0, PEN).T
            py = np.where(in_y, 0.0, PEN).T
            pz = np.where(in_z, 0.0, PEN).T
            semh = sem[hit].astype(np.float16)
            for (gg_, off, Mt, po, first, last, s0, pd) in units_of_slot[g]:
                a, b2 = s0, min(s0 + Mt, nh)
                if a >= nh:
                    continue
                w_ = b2 - a
                coef_m[0:10, off:off + w_] = chi[:, a:b2]
                coef_m[10:20, off:off + w_] = chi[:, a:b2]
                coef_m[20:30, off:off + w_] = clo[:, a:b2]
                coef_m[30:30 + BX, off:off + w_] = px[:, a:b2]
                coef_m[30 + BX:30 + BX + BY, off:off + w_] = py[:, a:b2]
                coef_m[30 + BX + BY:, off:off + w_] = pz[:, a:b2]
                ui = uindex[(gg_, off)]
                semp[po:po + w_, ui * C:(ui + 1) * C] = semh[a:b2]
        perm_blocks.append(my_blocks)
        coefx = np.concatenate([np.asarray(rhs, np.float32),
                                coef_m], axis=1).astype(ml_dtypes.bfloat16)
        in_maps.append({"COEF": coefx, "SEMP": semp})
    return in_maps, L_slots, (perm_blocks, grp_of, gg_of)


def kernel(**inputs):
    in_maps, L_slots, (perm_blocks, grp_of, gg_of) = _host_prep(**inputs)
    nc = _get_nc(L_slots)
    run = _get_runner(nc)
    results, _, _ = run(in_maps)
    out = np.empty((N, C), np.float32)
    lx = np.arange(GPTS) // (BY * BZ)
    ly = (np.arange(GPTS) // BZ) % BY
    lz = np.arange(GPTS) % BZ
    for ci in range(NCORES):
        o = results[ci]["OUT"].astype(np.float32)   # [OROWS, NGRP*OCOLS]
        for g in range(NSLOT):
            bid = perm_blocks[ci][g]
            x0 = (bid // (NBY * NBZ)) * BX
            y0 = ((bid // NBZ) % NBY) * BY
            z0 = (bid % NBZ) * BZ
            grp, gg = grp_of[g], gg_of[g]
            gi, s = gg // PERBANK, gg % PERBANK
            col0 = grp * OCOLS + s * GPTS
            blk = o[gi * 32:gi * 32 + C, col0:col0 + GPTS]   # [C, GPTS]
            gidx = ((x0 + lx) * W + (y0 + ly)) * D + (z0 + lz)
            out[gidx] = blk.T
    return out


# revision 35
# speedup vs baseline: 4.0194x; 1.0781x over previous
# Trainium2 Bass kernel for nn_LocalAggregator (Gaussian -> voxel-grid semantic
# compositing).
#
# Strategy: the voxel grid (60,60,36) is tiled into 1080 3-D blocks of
# (4,5,6) = 120 voxels.  Blocks are dealt to the 8 cores by sorted hit-count
# (rank 8k+c -> core c, slot k) so every core sees a near-identical load
# profile; the host un-permutes the output afterwards.  For each block the
# host builds the exact list of Gaussians whose integer box overlaps the
# block in all three dims.  In block-local coordinates the feature matrix
# (quadratic monomials + one-hot rows for the x/y/z box tests) is the SAME
# for every block, so a single small RHS is shared by all matmuls; all
# per-(block,Gaussian) data lives in the COEF matrix.  The E matmul runs in
# bf16 with a 3-way hi/lo product split (rows [hi,hi,lo] x features
# [fhi,flo,fhi] drop only the lo*lo term, ~1e-5 relative), Exp runs on the
# scalar engine over 12 units (3 PSUM banks x 4 blocks each) at a time, and
# the semantic reduction is an fp16 matmul accumulated into PSUM banks packed
# 12 blocks deep (3 partition stripes x 4 column slots) so one DVE copy +
# one Pool-engine DMA drains 12 blocks at once.  Dummy matmuls at t=0 ramp
# the PE clock to max p-state and a dummy activation preloads the Exp table.
import numpy as np
import ml_dtypes

H, W, D = 60, 60, 36
GRID = 0.08
SCALE_MULT = 3.0
P = 2048
C = 13
N = H * W * D                  # 129600
NCORES = 8
BX, BY, BZ = 4, 5, 6           # block shape
NBX, NBY, NBZ = H // BX, W // BY, D // BZ
NB = NBX * NBY * NBZ           # 1080 blocks total
GPTS = BX * BY * BZ            # 120 points per block
NSLOT = NB // NCORES           # 135 blocks (slots) per core
NPC = NSLOT * GPTS             # 16200 points per core
KF = 30 + BX + BY + BZ         # 45 feature rows (3x10 split products + onehots)
PEN = -2000.0                  # box-miss penalty (exp() == 0 in fp32)
PERBANK = 4                    # 120-col units per 512-f32 PSUM bank
NBANK = 2                      # banks per psE tile / act instruction
UPT = PERBANK * NBANK          # 12 units per psE tile
OGRP = 12                      # blocks per psO bank (3 stripes x 4 col slots)
OROWS = 77                     # 2*32+13 live partitions per output group
OCOLS = PERBANK * GPTS         # 480 cols per output group
NGRP = (NSLOT + OGRP - 1) // OGRP   # output groups per core (last partial)

_NC_CACHE: dict = {}
_JIT_CACHE: dict = {}


def _pack_units(L_slots):
    """units -> column-slots with partition stacking and load interleaving.

    Returns (cslots, grp_of, gg_of, gsz_of, Ltot) where cslots is a list of
    lists of (slot, coef_off, Mt, part_off, first_in_slot, last_in_slot, s0).
    COEF column offsets are assigned sequentially in processing order.
    Units with Mt<=64 stack 2-3 per column-slot at partition offsets
    {0, 32, 64} (matmul out/lhsT base partition constraint; offset 32 needs
    Mt<=32, 64 needs Mt<=64).  Packed column-slots are interleaved evenly
    between the single-unit ones so the PE load per activation tile stays
    flat.  psO groups are formed over the slot completion order.
    """
    singles, smalls = [], []
    for g, L in enumerate(L_slots):
        L = int(L)
        nparts = (L + 127) // 128
        part = []
        for s in range(0, L, 128):
            Mt = min(128, L - s)
            u = (g, Mt, s == 0, s + 128 >= L, s)
            if nparts > 1 or Mt > 64:
                part.append(u)
            else:
                smalls.append(u)
        if part:
            singles.append(part)      # parts of one slot stay adjacent
    # zero-padded pairs: both members padded to 64 rows; the out-matmul
    # then contracts the full 128 partitions at base 0 and the zero
    # semantics rows mask the partner block.
    multis = []
    for i in range(0, len(smalls) - 1, 2):
        multis.append([(smalls[i], 0, True), (smalls[i + 1], 64, True)])
    if len(smalls) % 2:
        multis.append([(smalls[-1], 0, False)])
    # interleave multis evenly among singles (keeping slot parts adjacent)
    seq = []          # list of cslots: list of ((g,Mt,first,last,s0), po)
    ns, nm = len(singles), len(multis)
    si = mi = 0
    acc = 0.0
    step = nm / max(1, ns + nm)
    while si < ns or mi < nm:
        acc += step
        if mi < nm and (acc >= 1.0 or si >= ns):
            seq.append(multis[mi]); mi += 1; acc -= 1.0
        elif si < ns:
            for u in singles[si]:
                seq.append([(u, 0, False)])
            si += 1
    # assign coef offsets in processing order; padded units reserve the
    # full 64 columns (zero coef -> E=0 -> w=1, zero semantics -> no-op)
    cslots = []
    cum = 0
    for cs in seq:
        out = []
        for (g, Mt, first, last, s0), po, padded in cs:
            w = 64 if padded else Mt
            out.append((g, cum, w, po, first, last, s0, padded))
            cum += w
        cslots.append(out)
    Ltot = cum
    # slot completion order -> psO groups
    comp = {}
    for ci, cs in enumerate(cslots):
        for (g, off, Mt, po, first, last, s0, pd) in cs:
            comp[g] = max(comp.get(g, -1), ci)
    order = sorted(range(len(L_slots)), key=lambda g: (comp[g], g))
    grp_of = [0] * len(L_slots)
    gg_of = [0] * len(L_slots)
    gsz_of = [0] * len(L_slots)
    nslot = len(L_slots)
    nfull = nslot // OGRP
    for pos, g in enumerate(order):
        q, rr = divmod(pos, OGRP)
        grp_of[g] = q
        gg_of[g] = rr
        gsz_of[g] = OGRP if q < nfull else nslot % OGRP
    return cslots, grp_of, gg_of, gsz_of, Ltot


def _build_nc(L_slots):
    import concourse.bacc as bacc
    import concourse.tile as tile
    from concourse import mybir

    L_slots = [int(x) for x in L_slots]
    cslots, grp_of, gg_of, gsz_of, Ltot = _pack_units(L_slots)
    units = [u for cs in cslots for u in cs]
    NU = len(units)
    NCS = len(cslots)
    tiles_u = [cslots[i:i + UPT] for i in range(0, NCS, UPT)]

    nc = bacc.Bacc("TRN2", target_bir_lowering=False, debug=False,
                   num_devices=NCORES)
    f32 = mybir.dt.float32
    bf16 = mybir.dt.bfloat16
    f16 = mybir.dt.float16
    COEF = nc.dram_tensor("COEF", [KF, GPTS + Ltot], bf16,
                          kind="ExternalInput")
    SEMP = nc.dram_tensor("SEMP", [128, NU * C], f16, kind="ExternalInput")
    OUT = nc.dram_tensor("OUT", [OROWS, NGRP * OCOLS], f16,
                         kind="ExternalOutput")

    usem = {}
    for ui_, u_ in enumerate(units):
        usem[(u_[0], u_[1])] = ui_

    # coef-chunk boundaries at flat unit indices (processing order)
    CH_U = [0, 13, 52, 100, NU]
    u_s0 = 52

    with tile.TileContext(nc) as tc:
        with (
            tc.tile_pool(name="big", bufs=1) as big_pool,
            tc.tile_pool(name="w", bufs=6) as w_pool,
            tc.tile_pool(name="og", bufs=4) as og_pool,
            tc.tile_pool(name="psE", bufs=3, space="PSUM") as pse_pool,
            tc.tile_pool(name="psO", bufs=2, space="PSUM") as pso_pool,
        ):
            coefx_b = big_pool.tile([KF, GPTS + Ltot], bf16)
            rhs_b = coefx_b[:, 0:GPTS]
            coef_b = coefx_b[:, GPTS:]
            semp_b = big_pool.tile([128, NU * C], f16)
            scr_b = big_pool.tile([1, 512], bf16)
            scr_o = big_pool.tile([1, 8], f16)

            # --- warmup: PE p-state ramp + activation table preload.
            # A cheap same-engine SEQ write seeds scr_b so the dummy
            # matmuls (which start the PE clock ramp) run immediately.
            nc.tensor.write(scr_b[0:1, 0:1],
                            np.zeros(1, ml_dtypes.bfloat16).tobytes())
            nc.scalar.activation(scr_o[0:1, 0:1], scr_b[0:1, 0:1],
                                 mybir.ActivationFunctionType.Exp)
            psD = pse_pool.tile([128, NBANK, 512], f32, name="psD", tag="psE")
            for _ in range(3):
                nc.tensor.matmul(psD[0:1, 0:1, 0:512], scr_b[0:1, 0:1],
                                 scr_b[0:1, 0:512], start=True, stop=True,
                                 skip_group_check=True)

            # --- input loads: all on SP (HWDGE); RHS rides with chunk 0
            cb = [units[u][1] if u < NU else Ltot for u in CH_U]
            nc.sync.dma_start(coefx_b[:, 0:GPTS + cb[1]],
                              COEF[:, 0:GPTS + cb[1]])
            nc.sync.dma_start(coef_b[:, cb[1]:cb[2]],
                              COEF[:, GPTS + cb[1]:GPTS + cb[2]])
            nc.sync.dma_start(semp_b[:, 0:u_s0 * C], SEMP[:, 0:u_s0 * C])
            nc.sync.dma_start(coef_b[:, cb[2]:cb[3]],
                              COEF[:, GPTS + cb[2]:GPTS + cb[3]])
            nc.sync.dma_start(semp_b[:, u_s0 * C:], SEMP[:, u_s0 * C:])
            nc.sync.dma_start(coef_b[:, cb[3]:cb[4]],
                              COEF[:, GPTS + cb[3]:GPTS + cb[4]])

            pso_t = {}    # group -> psO tile
            w_ts, psEs = {}, {}

            def emit_E(t):
                tu = tiles_u[t]
                psE = pse_pool.tile([128, NBANK, 512], f32, name=f"psE{t}",
                                    tag="psE")
                psEs[t] = psE
                for j, cs in enumerate(tu):
                    b, s = j // PERBANK, j % PERBANK
                    if len(cs) == 2 and cs[0][7]:
                        # padded pair: adjacent COEF columns -> one matmul
                        off0 = cs[0][1]
                        nc.tensor.matmul(
                            psE[0:128, b:b + 1, s * GPTS:(s + 1) * GPTS],
                            coef_b[:, off0:off0 + 128], rhs_b[:],
                            start=True, stop=True, skip_group_check=True)
                        continue
                    for (g, off, Mt, po, first, last, s0, pd) in cs:
                        nc.tensor.matmul(
                            psE[po:po + Mt, b:b + 1,
                                s * GPTS:(s + 1) * GPTS],
                            coef_b[:, off:off + Mt], rhs_b[:],
                            start=True, stop=True, skip_group_check=True)

            def emit_act(t):
                tu = tiles_u[t]
                psE = psEs[t]
                w_t = w_pool.tile([128, NBANK, OCOLS], f16, name=f"w{t}",
                                  tag="w")
                w_ts[t] = w_t
                mtmax = max(u[3] + u[2] for cs in tu for u in cs)
                nb_full, rem = divmod(len(tu), PERBANK)
                if nb_full:
                    nc.scalar.activation(
                        w_t[0:mtmax, 0:nb_full, :],
                        psE[0:mtmax, 0:nb_full, 0:OCOLS],
                        mybir.ActivationFunctionType.Exp)
                if rem:
                    nc.scalar.activation(
                        w_t[0:mtmax, nb_full:nb_full + 1, 0:rem * GPTS],
                        psE[0:mtmax, nb_full:nb_full + 1, 0:rem * GPTS],
                        mybir.ActivationFunctionType.Exp)

            def emit_out(t):
                tu = tiles_u[t]
                w_t = w_ts[t]
                for j, cs in enumerate(tu):
                  b, js = j // PERBANK, j % PERBANK
                  for (g, off, Mt, po, first, last, s0, pd) in cs:
                    grp, gg, gsz = grp_of[g], gg_of[g], gsz_of[g]
                    gi, s = gg // PERBANK, gg % PERBANK
                    ui = usem[(g, off)]
                    if first and gg == 0:
                        pso_t[grp] = pso_pool.tile([OROWS, OCOLS], f32,
                                                   name=f"psO{grp}",
                                                   tag="psO")
                    k0, k1 = (0, 128) if pd else (0, Mt)
                    nc.tensor.matmul(
                        pso_t[grp][gi * 32:gi * 32 + C,
                                   s * GPTS:(s + 1) * GPTS],
                        semp_b[k0:k1, ui * C:(ui + 1) * C],
                        w_t[k0:k1, b:b + 1,
                            js * GPTS:(js + 1) * GPTS],
                        start=first, stop=last, skip_group_check=True)
                    if last and gg == gsz - 1:
                        top = gg // PERBANK            # last stripe index
                        grows = top * 32 + C
                        gcols = OCOLS if top > 0 else gsz * GPTS
                        outg = og_pool.tile([OROWS, OCOLS], f16,
                                            name=f"og{grp}", tag="og")
                        nc.vector.tensor_copy(outg[0:grows, 0:gcols],
                                              pso_t[grp][0:grows, 0:gcols])
                        dma_eng = nc.gpsimd if grp < NGRP - 2 else nc.sync
                        dma_eng.dma_start(
                            OUT[0:grows, grp * OCOLS:grp * OCOLS + gcols],
                            outg[0:grows, 0:gcols])

            emit_E(0)
            for t in range(len(tiles_u)):
                emit_act(t)
                if t + 1 < len(tiles_u):
                    emit_E(t + 1)
                emit_out(t)
    nc.compile()
    return nc


def _get_nc(L_slots):
    key = tuple(int(x) for x in L_slots)
    if key not in _NC_CACHE:
        _NC_CACHE[key] = _build_nc(L_slots)
    return _NC_CACHE[key]


def _get_runner(nc):
    """Cached shard_map-jitted executor for one Bass program (axon/PJRT path).

    Mirrors concourse.bass2jax.run_bass_via_pjrt but keeps the jitted callable
    so repeated runs don't rebuild/recompile."""
    if id(nc) in _JIT_CACHE:
        return _JIT_CACHE[id(nc)]
    import jax
    from concourse import bass2jax, mybir
    from jax.experimental.shard_map import shard_map
    from jax.sharding import Mesh, PartitionSpec

    bass2jax.install_neuronx_cc_hook()
    partition_name = (nc.partition_id_tensor.name
                      if nc.partition_id_tensor else None)
    in_names, out_names, out_avals, zero_outs = [], [], [], []
    for alloc in nc.m.functions[0].allocations:
        if not isinstance(alloc, mybir.MemoryLocationSet):
            continue
        name = alloc.memorylocations[0].name
        if alloc.kind == "ExternalInput":
            if name == partition_name:
                continue
            in_names.append(name)
        elif alloc.kind == "ExternalOutput":
            shape = tuple(alloc.tensor_shape)
            dtype = mybir.dt.np(alloc.dtype)
            out_names.append(name)
            out_avals.append(jax.core.ShapedArray(shape, dtype))
            zero_outs.append(np.zeros(shape, dtype))
    n_params = len(in_names)
    all_in_names = in_names + out_names
    if partition_name is not None:
        all_in_names = all_in_names + [partition_name]

    def _body(*args):
        operands = list(args)
        if partition_name is not None:
            operands.append(bass2jax.partition_id_tensor())
        outs = bass2jax._bass_exec_p.bind(
            *operands,
            out_avals=tuple(out_avals),
            in_names=tuple(all_in_names),
            out_names=tuple(out_names),
            lowering_input_output_aliases=(),
            sim_require_finite=True,
            sim_require_nnan=True,
            nc=nc,
        )
        return tuple(outs)

    devices = jax.devices()[:NCORES]
    mesh = Mesh(np.asarray(devices), ("core",))
    donate = tuple(range(n_params, n_params + len(out_names)))
    sharded = jax.jit(
        shard_map(_body, mesh=mesh,
                  in_specs=(PartitionSpec("core"),) * (n_params + len(out_names)),
                  out_specs=(PartitionSpec("core"),) * len(out_names),
                  check_rep=False),
        donate_argnums=donate, keep_unused=True)

    def run(in_maps, rounds=1):
        concat_in = [np.concatenate([np.asarray(m[nm]) for m in in_maps], axis=0)
                     for nm in in_names]
        outs = None
        for _ in range(rounds):
            zo = [np.concatenate([z] * NCORES, axis=0) for z in zero_outs]
            outs = sharded(*concat_in, *zo)
        outs = [np.asarray(o) for o in outs]
        results = []
        for ci in range(NCORES):
            d = {}
            for oi, nm in enumerate(out_names):
                per = outs[oi].shape[0] // NCORES
                d[nm] = outs[oi][ci * per:(ci + 1) * per]
            results.append(d)
        return results, sharded, (concat_in, zero_outs, in_names, out_names)

    sharded_nd = jax.jit(
        shard_map(_body, mesh=mesh,
                  in_specs=(PartitionSpec("core"),) * (n_params + len(out_names)),
                  out_specs=(PartitionSpec("core"),) * len(out_names),
                  check_rep=False),
        keep_unused=True)

    def timeit(in_maps, iters=30):
        import time as _time
        from jax.sharding import NamedSharding
        sh = NamedSharding(mesh, PartitionSpec("core"))
        concat_in = [np.concatenate([np.asarray(m[nm]) for m in in_maps], axis=0)
                     for nm in in_names]
        zo = [np.concatenate([z] * NCORES, axis=0) for z in zero_outs]
        args = [jax.device_put(a, sh) for a in concat_in + zo]
        outs = sharded_nd(*args)
        jax.block_until_ready(outs)
        t0 = _time.time()
        for _ in range(iters):
            outs = sharded_nd(*args)
        jax.block_until_ready(outs)
        return (_time.time() - t0) / iters

    run.timeit = timeit
    _JIT_CACHE[id(nc)] = run
    return run


def _bf16_split(x):
    hi = x.astype(ml_dtypes.bfloat16).astype(np.float32)
    return hi, (x - hi).astype(np.float32)


def _host_prep(pts, means3D, opacities, semantics, scales, cov3D, origin_use):
    pts = np.asarray(pts, np.float32).reshape(N, 3)
    mu32 = np.asarray(means3D, np.float32).reshape(P, 3)
    op = np.asarray(opacities, np.float64).reshape(P)
    sem = np.asarray(semantics, np.float32).reshape(P, C)
    sc32 = np.asarray(scales, np.float32).reshape(P, 3)
    cov = np.asarray(cov3D, np.float64).reshape(P, 3, 3)
    org32 = np.asarray(origin_use, np.float32).reshape(3)

    # --- integer binning, replicated in fp32 exactly like the reference ---
    radii = np.ceil(sc32.max(-1) * np.float32(SCALE_MULT) / np.float32(GRID)
                    ).astype(np.int32).astype(np.int64)
    m_int = ((mu32 - org32) / np.float32(GRID)).astype(np.int32).astype(np.int64)
    p_int = ((pts - org32) / np.float32(GRID)).astype(np.int32).astype(np.int64)

    # structured-input check: points must be the (i, j, k) voxel-center grid
    idx = np.arange(N)
    kk = idx % D
    col = idx // D
    jj = col % W
    ii = col // W
    grid_int = np.stack([ii, jj, kk], axis=-1)
    if not np.array_equal(p_int, grid_int):
        raise RuntimeError("kernel: unstructured pts not supported by fast path")

    # --- per-Gaussian inverse covariance (float64) ---
    a, b, c_, d, e, f = (cov[:, 0, 0], cov[:, 1, 1], cov[:, 2, 2],
                         cov[:, 0, 1], cov[:, 1, 2], cov[:, 0, 2])
    det = a * (b * c_ - e * e) - d * (d * c_ - e * f) + f * (d * e - b * f)
    ixx = (b * c_ - e * e) / det
    iyy = (a * c_ - f * f) / det
    izz = (a * b - d * d) / det
    ixy = (e * f - d * c_) / det
    iyz = (d * f - a * e) / det
    ixz = (d * e - b * f) / det
    A = np.empty((P, 3, 3))
    A[:, 0, 0], A[:, 1, 1], A[:, 2, 2] = ixx, iyy, izz
    A[:, 0, 1] = A[:, 1, 0] = ixy
    A[:, 1, 2] = A[:, 2, 1] = iyz
    A[:, 0, 2] = A[:, 2, 0] = ixz
    mu = mu32.astype(np.float64)
    logop = np.log(op)

    # --- blocks: bounds, hit lists, load-balanced assignment ---
    mx, my, mz = m_int[:, 0], m_int[:, 1], m_int[:, 2]
    blocks = []            # (x0, y0, z0)
    hits = []
    for bxi in range(NBX):
        for byi in range(NBY):
            for bzi in range(NBZ):
                x0, y0, z0 = bxi * BX, byi * BY, bzi * BZ
                hit = np.where(
                    (mx + radii >= x0) & (mx - radii <= x0 + BX - 1) &
                    (my + radii >= y0) & (my - radii <= y0 + BY - 1) &
                    (mz + radii >= z0) & (mz - radii <= z0 + BZ - 1))[0]
                blocks.append((x0, y0, z0))
                hits.append(hit)
    Ls = np.array([len(h) for h in hits])
    order = np.argsort(-Ls, kind="stable")     # rank r -> block id
    # core c, slot k gets block order[8k + c]; slot size = L of rank 8k
    L_slots = [max(1, int(Ls[order[8 * k]])) for k in range(NSLOT)]
    offs = np.concatenate([[0], np.cumsum(L_slots)]).astype(int)
    Ltot = int(offs[-1])
    cslots, grp_of, gg_of, gsz_of, Ltot = _pack_units(L_slots)
    units = [u for cs in cslots for u in cs]
    NU = len(units)

    # --- shared RHS: block-local features (hi/lo pairs) + one-hot rows ---
    lx = np.arange(GPTS) // (BY * BZ)
    ly = (np.arange(GPTS) // BZ) % BY
    lz = np.arange(GPTS) % BZ
    xi = ((lx - (BX - 1) / 2.0) * GRID).astype(np.float32)
    yi = ((ly - (BY - 1) / 2.0) * GRID).astype(np.float32)
    zi = ((lz - (BZ - 1) / 2.0) * GRID).astype(np.float32)
    feat10 = np.stack([xi * xi, yi * yi, zi * zi, xi * yi, yi * zi, xi * zi,
                       xi, yi, zi, np.ones(GPTS, np.float32)])
    fhi, flo = _bf16_split(feat10)
    rhs = np.zeros((KF, GPTS), np.float32)
    rhs[0:10] = fhi
    rhs[10:20] = flo
    rhs[20:30] = fhi
    rhs[30 + lx, np.arange(GPTS)] = 1.0
    rhs[30 + BX + ly, np.arange(GPTS)] = 1.0
    rhs[30 + BX + BY + lz, np.arange(GPTS)] = 1.0
    rhs = rhs.astype(ml_dtypes.bfloat16)

    # --- per-core COEF / SEMP ---
    in_maps = []
    perm_blocks = []       # per core: slot -> block id
    # per-slot units (in processing order they appear)
    units_of_slot = {}
    for u in units:
        units_of_slot.setdefault(u[0], []).append(u)
    for ci in range(NCORES):
        coef_m = np.zeros((KF, Ltot), np.float32)
        semp = np.zeros((128, NU * C), np.float16)
        my_blocks = []
        uindex = {}
        for ui, u in enumerate(units):
            uindex[(u[0], u[1])] = ui
        for g in range(NSLOT):
            bid = int(order[8 * g + ci])
            my_blocks.append(bid)
            hit = hits[bid]
            nh = len(hit)
            if nh == 0:
                continue
            x0, y0, z0 = blocks[bid]
            cx = (x0 + (BX - 1) / 2.0 + 0.5) * GRID
            cy = (y0 + (BY - 1) / 2.0 + 0.5) * GRID
            cz = (z0 + (BZ - 1) / 2.0 + 0.5) * GRID
            dmu = mu[hit] - np.array([cx, cy, cz])      # [nh, 3]
            Ah = A[hit]                                  # [nh, 3, 3]
            Amu = np.einsum('pij,pj->pi', Ah, dmu)
            muAmu = np.einsum('pi,pi->p', dmu, Amu)
            c10 = np.stack([
                -0.5 * Ah[:, 0, 0], -0.5 * Ah[:, 1, 1], -0.5 * Ah[:, 2, 2],
                -Ah[:, 0, 1], -Ah[:, 1, 2], -Ah[:, 0, 2],
                Amu[:, 0], Amu[:, 1], Amu[:, 2],
                -0.5 * muAmu + logop[hit]]).astype(np.float32)
            chi, clo = _bf16_split(c10)
            in_x = (np.abs(x0 + np.arange(BX)[None, :] - mx[hit, None])
                    <= radii[hit, None])
            in_y = (np.abs(y0 + np.arange(BY)[None, :] - my[hit, None])
                    <= radii[hit, None])
            in_z = (np.abs(z0 + np.arange(BZ)[None, :] - mz[hit, None])
                    <= radii[hit, None])
            px = np.where(in_x, 0.0, PEN).T
            py = np.where(in_y, 0.0, PEN).T
            pz = np.where(in_z, 0.0, PEN).T
            semh = sem[hit].astype(np.float16)
            for (gg_, off, Mt, po, first, last, s0, pd) in units_of_slot[g]:
                a, b2 = s0, min(s0 + Mt, nh)
                if a >= nh:
                    continue
                w_ = b2 - a
                coef_m[0:10, off:off + w_] = chi[:, a:b2]
                coef_m[10:20, off:off + w_] = chi[:, a:b2]
                coef_m[20:30, off:off + w_] = clo[:, a:b2]
                coef_m[30:30 + BX, off:off + w_] = px[:, a:b2]
                coef_m[30 + BX:30 + BX + BY, off:off + w_] = py[:, a:b2]
                coef_m[30 + BX + BY:, off:off + w_] = pz[:, a:b2]
                ui = uindex[(gg_, off)]
                semp[po:po + w_, ui * C:(ui + 1) * C] = semh[a:b2]
        perm_blocks.append(my_blocks)
        coefx = np.concatenate([np.asarray(rhs, np.float32),
                                coef_m], axis=1).astype(ml_dtypes.bfloat16)
        in_maps.append({"COEF": coefx, "SEMP": semp})
    return in_maps, L_slots, (perm_blocks, grp_of, gg_of)


def kernel(**inputs):
    in_maps, L_slots, (perm_blocks, grp_of, gg_of) = _host_prep(**inputs)
    nc = _get_nc(L_slots)
    run = _get_runner(nc)
    results, _, _ = run(in_maps)
    out = np.empty((N, C), np.float32)
    lx = np.arange(GPTS) // (BY * BZ)
    ly = (np.arange(GPTS) // BZ) % BY
    lz = np.arange(GPTS) % BZ
    for ci in range(NCORES):
        o = results[ci]["OUT"].astype(np.float32)   # [OROWS, NGRP*OCOLS]
        for g in range(NSLOT):
            bid = perm_blocks[ci][g]
            x0 = (bid // (NBY * NBZ)) * BX
            y0 = ((bid // NBZ) % NBY) * BY
            z0 = (bid % NBZ) * BZ
            grp, gg = grp_of[g], gg_of[g]
            gi, s = gg // PERBANK, gg % PERBANK
            col0 = grp * OCOLS + s * GPTS
            blk = o[gi * 32:gi * 32 + C, col0:col0 + GPTS]   # [C, GPTS]
            gidx = ((x0 + lx) * W + (y0 + ly)) * D + (z0 + lz)
            out[gidx] = blk.T
    return out
